# revision 1
# baseline (speedup 1.0000x reference)
"""Builder for the CausalWanModel sparse-attention TRN2 kernel.

Sharding (8 cores, 12 heads of HD=128):
  pair p in {0,1,2,3} owns heads {3p, 3p+1, 3p+2}; core 2p ("A") has
  slot0 = head 3p, core 2p+1 ("B") has slot0 = head 3p+2; both share
  slot1 = head 3p+1, split by attention window position: A covers
  cache[0:split] + all new tokens, B covers cache[split:L0]  (split =
  L1 - NEW so both see L1 "part1" keys; A's part1 is zero-padded and
  masked via exp bias).  The SPMD program is identical on all cores;
  only input data differs.

Collectives (all tiny, latency hidden behind compute):
  c1: AllReduce[all 8] of masked q sum-of-squares  (RMSNorm factors)
  c2: AllReduce[all 8] of masked k sum-of-squares
  c3: AllReduce[pairs] of slot1 softmax denominators

Attention uses transposed scores [s, t] so PV consumes exp tiles as the
moving operand directly (no P transpose); softmax skips max-subtraction
(|scores| <= sqrt(HD) after RMSNorm); denominators accumulate on DVE and
are partition-reduced with a ones-vector matmul.
"""

import math
import contextlib
import numpy as np

import concourse.bass as bass
import concourse.tile as tile
from concourse import bacc, mybir
from concourse.masks import make_identity

F32 = mybir.dt.float32
F32R = mybir.dt.float32r
AF = mybir.ActivationFunctionType
ALU = mybir.AluOpType

EPS = 1e-6
NEG_BIAS = -60.0  # exp(x + NEG_BIAS) ~ 0 for masked lanes


def subchunks(total, size=128):
    out = []
    off = 0
    while off < total:
        out.append((off, min(size, total - off)))
        off += size
    return out


def full_cfg():
    return dict(T=1560, NT=4, XD=1536, D=128, L0=7800, L1=4680, SUPER=512)


def build_program(cfg, n_cores=8):
    T, XD, D = cfg["T"], cfg["XD"], cfg["D"]
    NT = cfg["NT"]
    TC = T // NT
    assert TC * NT == T
    NK = XD // 128
    L0, L1, SUPER = cfg["L0"], cfg["L1"], cfg["SUPER"]
    NEW = T
    NJ = NK
    inv_sqrt_d = 1.0 / math.sqrt(D)
    new_subs = subchunks(NEW)
    n_new = len(new_subs)
    n_sub1 = len(subchunks(L1))

    nc = bacc.Bacc("TRN2", target_bir_lowering=False, debug=False,
                   num_devices=n_cores)

    def din(name, shape, dt=F32R):
        return nc.dram_tensor(name, shape, dt, kind="ExternalInput")

    xT_d = din("xT", [XD, T])
    w_d = {"q": din("wq", [XD, 256]), "k": din("wk", [XD, 256]),
           "v": din("wv", [XD, 256])}
    woT_d = din("woT", [256, XD])
    b_d = {"q": din("bq", [256, 1], F32), "k": din("bk", [256, 1], F32),
           "v": din("bv", [256, 1], F32)}
    g_d = {"q": din("gq", [256, 1], F32), "k": din("gk", [256, 1], F32)}
    bo_d = din("bo", [XD, 1], F32)
    cos2_d = din("cos2", [128, T], F32)
    sin2_d = din("sin2", [128, T], F32)
    sqmask_d = din("sqmask", [256, 1])
    kc0_d = din("kc0T", [128, L0])
    vc0_d = din("vc0", [L0, 128])
    kc1_d = din("kc1T", [128, L1])
    vc1_d = din("vc1", [L1, 128])
    ident_d = din("identc", [128, 128])
    swp_d = din("swpc", [128, 128])
    bias1_d = din("bias1", [128, n_sub1], F32)
    bias2_d = din("bias2", [128, n_new], F32)
    yT_d = nc.dram_tensor("yT", [XD, T], F32, kind="ExternalOutput")

    with tile.TileContext(nc) as tc, contextlib.ExitStack() as ctx:
        const = ctx.enter_context(tc.tile_pool(name="const", bufs=1))
        persist = ctx.enter_context(tc.tile_pool(name="persist", bufs=1))
        dram = ctx.enter_context(tc.tile_pool(name="dram", bufs=1, space="DRAM"))

        # ---- constants (host-built; memset/affine_select on f32r is
        # rejected by walrus ISA checks) ----
        ident = const.tile([128, 128], F32R)
        nc.sync.dma_start(ident[:], ident_d.ap())
        swp = const.tile([128, 128], F32R)
        nc.sync.dma_start(swp[:], swp_d.ap())
        ones_f32 = const.tile([128, 1], F32)
        nc.vector.memset(ones_f32[:], 1.0)
        eps_c = const.tile([1, 1], F32)
        nc.vector.memset(eps_c[:], EPS)

        # ---- small input tensors ----
        biases = const.tile([128, 6], F32)
        for i, nm in enumerate(("q", "k", "v")):
            nc.sync.dma_start(biases[:, 2 * i:2 * i + 1], b_d[nm].ap()[0:128, :])
            nc.sync.dma_start(biases[:, 2 * i + 1:2 * i + 2],
                              b_d[nm].ap()[128:256, :])
        gs = const.tile([128, 4], F32)
        for i, nm in enumerate(("q", "k")):
            nc.sync.dma_start(gs[:, 2 * i:2 * i + 1], g_d[nm].ap()[0:128, :])
            nc.sync.dma_start(gs[:, 2 * i + 1:2 * i + 2], g_d[nm].ap()[128:256, :])
        bo8 = const.tile([128, NJ], F32)
        bo_sb = const.tile([128, NJ], F32)
        nc.sync.dma_start(bo_sb[:].rearrange("p (j o) -> p j o", j=NJ),
                          bo_d.ap().rearrange("(j p) o -> p j o", p=128))
        nc.vector.tensor_scalar_mul(bo8[:], bo_sb[:], 1.0 / n_cores)
        sqmask = const.tile([128, 2], F32R)
        nc.sync.dma_start(sqmask[:, 0:1], sqmask_d.ap()[0:128, :])
        nc.sync.dma_start(sqmask[:, 1:2], sqmask_d.ap()[128:256, :])
        bias1 = const.tile([128, n_sub1], F32)
        nc.sync.dma_start(bias1[:], bias1_d.ap())
        bias2 = const.tile([128, n_new], F32)
        nc.sync.dma_start(bias2[:], bias2_d.ap())
        # collective bounce buffers
        cbuf = {}
        for nm in ("sqq", "sqk", "den1"):
            cin = dram.tile([1, T], F32, tag=f"cin_{nm}", name=f"cin_{nm}")
            cout = dram.tile([1, T], F32, tag=f"cout_{nm}", name=f"cout_{nm}")
            cbuf[nm] = (cin, cout)
        groups_all = [list(range(n_cores))]
        groups_pair = [[i, i + 1] for i in range(0, n_cores, 2)]

        # persistent across phases
        sq_sb = {nm: persist.tile([1, T], F32, tag=f"sq{nm}", name=f"sq{nm}") for nm in ("q", "k")}
        qkf = {nm: [persist.tile([128, T], F32R, tag=f"f{nm}{cc}", name=f"f{nm}{cc}")
                    for cc in range(2)] for nm in ("q", "k")}
        v_nat = [persist.tile([128, n_new * 128], F32R, tag=f"vn{cc}", name=f"vn{cc}")
                 for cc in range(2)]
        acc = [persist.tile([128, T], F32, tag=f"acc{s}", name=f"acc{s}") for s in range(2)]
        out1_sb = persist.tile([128, T], F32, tag="out1sb")
        den_sb = [persist.tile([1, T], F32, tag=f"den{s}", name=f"den{s}") for s in range(2)]

        # ================= P1/P2: projections + sumsq =================
        with tc.tile_pool(name="mid", bufs=1) as mid:
            xp = {nm: [mid.tile([128, T], F32R if nm == "v" else F32,
                                tag=f"x{nm}{cc}", name=f"x{nm}{cc}") for cc in range(2)]
                  for nm in ("q", "k", "v")}
            with tc.tile_pool(name="xT", bufs=1) as xpool, \
                 tc.tile_pool(name="wstr", bufs=4) as wpool, \
                 tc.tile_pool(name="proj_ps", bufs=1, space="PSUM") as pps, \
                 tc.tile_pool(name="sq_ps", bufs=1, space="PSUM") as sps, \
                 tc.tile_pool(name="sqt", bufs=3) as sqt_pool:

                xT_sb = xpool.tile([128, NK * T], F32R)
                nc.sync.dma_start(
                    xT_sb[:].rearrange("p (k t) -> p k t", k=NK),
                    xT_d.ap().rearrange("(k p) t -> p k t", p=128))

                for nm in ("q", "k", "v"):
                    for cc in range(2):
                        ps = [pps.tile([128, TC], F32, tag=f"proj{t}", name=f"proj{t}")
                              for t in range(NT)]
                        for kc in range(NK):
                            wt = wpool.tile([128, 128], F32R, tag="w")
                            nc.sync.dma_start(
                                wt[:], w_d[nm].ap()[kc * 128:(kc + 1) * 128,
                                                    cc * 128:(cc + 1) * 128])
                            for t in range(NT):
                                nc.tensor.matmul(
                                    ps[t][:], wt[:],
                                    xT_sb[:, kc * T + t * TC:kc * T + (t + 1) * TC],
                                    start=(kc == 0), stop=(kc == NK - 1))
                        ib = ("q", "k", "v").index(nm)
                        for t in range(NT):
                            nc.vector.tensor_scalar_add(
                                xp[nm][cc][:, t * TC:(t + 1) * TC], ps[t][:],
                                biases[:, 2 * ib + cc:2 * ib + cc + 1])
                    if nm in ("q", "k"):
                        for t in range(NT):
                            qps = sps.tile([1, TC], F32, tag="sqps")
                            for cc in range(2):
                                sqt = sqt_pool.tile([128, TC], F32R, tag="sqt")
                                nc.vector.tensor_tensor(
                                    out=sqt[:],
                                    in0=xp[nm][cc][:, t * TC:(t + 1) * TC],
                                    in1=xp[nm][cc][:, t * TC:(t + 1) * TC],
                                    op=ALU.mult)
                                nc.tensor.matmul(qps[:], sqmask[:, cc:cc + 1],
                                                 sqt[:], start=(cc == 0),
                                                 stop=(cc == 1))
                            nc.vector.tensor_copy(
                                sq_sb[nm][:, t * TC:(t + 1) * TC], qps[:])
                        key = "sq" + nm
                        nc.gpsimd.dma_start(cbuf[key][0][:], sq_sb[nm][:])
                        nc.gpsimd.collective_compute(
                            "AllReduce", ALU.add, replica_groups=groups_all,
                            ins=[cbuf[key][0].opt()], outs=[cbuf[key][1].opt()])

            # ================= P3/P4/P5: rms, rope, v-transpose ========
            with tc.tile_pool(name="rope", bufs=2) as rp, \
                 tc.tile_pool(name="ropec", bufs=1) as rpc, \
                 tc.tile_pool(name="rope_ps", bufs=3, space="PSUM") as rps:
                cos2 = rpc.tile([128, T], F32)
                sin2 = rpc.tile([128, T], F32)
                nc.sync.dma_start(cos2[:], cos2_d.ap())
                nc.sync.dma_start(sin2[:], sin2_d.ap())
                rms_b = {}
                for nm in ("q", "k"):
                    tot = rp.tile([1, T], F32, tag="rtmp", name="rtot")
                    nc.sync.dma_start(tot[:], cbuf["sq" + nm][1][:])
                    srt = rp.tile([1, T], F32, tag="rtmp", name="rsrt")
                    nc.scalar.activation(srt[:], tot[:], AF.Sqrt,
                                         bias=eps_c[:], scale=1.0 / XD)
                    rec = rp.tile([1, T], F32, tag="rtmp", name="rrec")
                    nc.vector.reciprocal(rec[:], srt[:])
                    if nm == "q":
                        rec2 = rp.tile([1, T], F32, tag="rtmp", name="rrec2")
                        nc.vector.tensor_scalar_mul(rec2[:], rec[:], inv_sqrt_d)
                        rec = rec2
                    rms_b[nm] = rp.tile([128, T], F32, tag=f"rms{nm}", name=f"rms{nm}", bufs=1)
                    nc.gpsimd.partition_broadcast(rms_b[nm][:], rec[0:1, :])

                for ig, nm in enumerate(("q", "k")):
                    for cc in range(2):
                        xg = rp.tile([128, T], F32R, tag="xg")
                        nc.vector.tensor_scalar_mul(
                            xg[:], xp[nm][cc][:],
                            gs[:, 2 * ig + cc:2 * ig + cc + 1])
                        m1 = rp.tile([128, T], F32, tag="m1")
                        nc.vector.tensor_tensor(out=m1[:], in0=xg[:],
                                                in1=cos2[:], op=ALU.mult)
                        rq = rp.tile([128, T], F32, tag="rq")
                        for t in range(NT):
                            swps = rps.tile([128, TC], F32, tag="swp")
                            nc.tensor.matmul(swps[:], swp[:],
                                             xg[:, t * TC:(t + 1) * TC],
                                             start=True, stop=True)
                            m2 = rp.tile([128, TC], F32, tag="m2")
                            nc.vector.tensor_tensor(
                                out=m2[:], in0=swps[:],
                                in1=sin2[:, t * TC:(t + 1) * TC], op=ALU.mult)
                            nc.vector.tensor_tensor(
                                out=rq[:, t * TC:(t + 1) * TC],
                                in0=m1[:, t * TC:(t + 1) * TC], in1=m2[:],
                                op=ALU.add)
                        nc.vector.tensor_tensor(out=qkf[nm][cc][:], in0=rq[:],
                                                in1=rms_b[nm][:], op=ALU.mult)

                for cc in range(2):
                    for j, (off, ck) in enumerate(new_subs):
                        tp = rps.tile([128, 128], F32R, tag="vtp")
                        nc.tensor.matmul(tp[0:ck, :],
                                         xp["v"][cc][:, off:off + ck], ident[:],
                                         is_transpose=True, start=True, stop=True)
                        nc.vector.tensor_copy(
                            v_nat[cc][0:ck, j * 128:(j + 1) * 128], tp[0:ck, :])

        # ================= P6/P7: attention =================
        with tc.tile_pool(name="outps", bufs=1, space="PSUM") as ops, \
             tc.tile_pool(name="dps", bufs=1, space="PSUM") as dpool:

            def attn_chunk(slot, k_ap, v_ap, ck, btile, bidx, first, last,
                           out_ps, ep_, stp):
                for t in range(NT):
                    st = stp.tile([128, TC], F32, tag="st")
                    nc.tensor.matmul(st[0:ck, :], k_ap,
                                     qkf["q"][slot][:, t * TC:(t + 1) * TC],
                                     start=True, stop=True)
                    ex = ep_.tile([128, TC], F32R, tag="ex")
                    bias = 0.0 if btile is None else btile[0:ck, bidx:bidx + 1]
                    nc.scalar.activation(ex[0:ck, :], st[0:ck, :], AF.Exp,
                                         bias=bias)
                    if first:
                        assert ck == 128
                        nc.vector.tensor_copy(acc[slot][:, t * TC:(t + 1) * TC],
                                              ex[:, :])
                    else:
                        nc.vector.tensor_add(
                            acc[slot][0:ck, t * TC:(t + 1) * TC],
                            acc[slot][0:ck, t * TC:(t + 1) * TC], ex[0:ck, :])
                    nc.tensor.matmul(out_ps[t][:], v_ap, ex[0:ck, :],
                                     start=first, stop=last,
                                     skip_group_check=True)

            def attend(slot, segments, out_ps, evac_to):
                gidx = 0
                total = sum(len(subchunks(L)) for _, _, _, L, _ in segments)
                with tc.tile_pool(name=f"att{slot}", bufs=3) as ap_, \
                     tc.tile_pool(name=f"exp{slot}", bufs=4) as ep_, \
                     tc.tile_pool(name=f"st{slot}", bufs=2, space="PSUM") as stp:
                    for kind, ksrc, vsrc, L, btile in segments:
                        if kind == "dram":
                            for soff, ssz in subchunks(L, SUPER):
                                ks = ap_.tile([128, SUPER], F32R, tag="ks")
                                nc.sync.dma_start(ks[:, 0:ssz],
                                                  ksrc.ap()[:, soff:soff + ssz])
                                vs = ap_.tile([128, SUPER], F32R, tag="vs")
                                subs = subchunks(ssz)
                                for j, (o2, c2) in enumerate(subs):
                                    nc.sync.dma_start(
                                        vs[0:c2, j * 128:(j + 1) * 128],
                                        vsrc.ap()[soff + o2:soff + o2 + c2, :])
                                for j, (o2, c2) in enumerate(subs):
                                    attn_chunk(slot, ks[:, o2:o2 + c2],
                                               vs[0:c2, j * 128:(j + 1) * 128],
                                               c2, btile, (soff + o2) // 128,
                                               gidx == 0, gidx == total - 1,
                                               out_ps, ep_, stp)
                                    gidx += 1
                        else:
                            for j, (off, ck) in enumerate(subchunks(L)):
                                attn_chunk(slot, ksrc[:, off:off + ck],
                                           v_nat[slot][0:ck, j * 128:(j + 1) * 128],
                                           ck, btile, j,
                                           gidx == 0, gidx == total - 1,
                                           out_ps, ep_, stp)
                                gidx += 1
                for t in range(NT):
                    dps = dpool.tile([1, TC], F32, tag="dps")
                    nc.tensor.matmul(dps[:], ones_f32[:],
                                     acc[slot][:, t * TC:(t + 1) * TC],
                                     start=True, stop=True)
                    nc.vector.tensor_copy(den_sb[slot][:, t * TC:(t + 1) * TC],
                                          dps[:])
                if evac_to is not None:
                    for t in range(NT):
                        nc.scalar.copy(evac_to[:, t * TC:(t + 1) * TC],
                                       out_ps[t][:])

            out1_ps = [ops.tile([128, TC], F32, tag=f"o_{t}", name=f"o1_{t}") for t in range(NT)]
            attend(1, [("dram", kc1_d, vc1_d, L1, bias1),
                       ("sbuf", qkf["k"][1], None, NEW, bias2)],
                   out1_ps, out1_sb)
            nc.gpsimd.dma_start(cbuf["den1"][0][:], den_sb[1][:])
            nc.gpsimd.collective_compute(
                "AllReduce", ALU.add, replica_groups=groups_pair,
                ins=[cbuf["den1"][0].opt()], outs=[cbuf["den1"][1].opt()])

            out0_ps = [ops.tile([128, TC], F32, tag=f"o_{t}", name=f"o0_{t}") for t in range(NT)]
            attend(0, [("dram", kc0_d, vc0_d, L0, None),
                       ("sbuf", qkf["k"][0], None, NEW, None)],
                   out0_ps, None)

            # ================= P8: normalize + out projection ==========
            with tc.tile_pool(name="fin", bufs=2) as fp_, \
                 tc.tile_pool(name="yps", bufs=3, space="PSUM") as yps:
                woT_sb = fp_.tile([128, 2 * XD], F32R, bufs=1)
                nc.sync.dma_start(woT_sb[:, 0:XD], woT_d.ap()[0:128, :])
                nc.sync.dma_start(woT_sb[:, XD:2 * XD], woT_d.ap()[128:256, :])
                den1_tot = fp_.tile([1, T], F32, tag="d1t", bufs=1)
                nc.sync.dma_start(den1_tot[:], cbuf["den1"][1][:])
                recips = []
                for slot, dsrc in ((0, den_sb[0]), (1, den1_tot)):
                    rc = fp_.tile([1, T], F32, tag=f"rc{slot}", name=f"rc{slot}", bufs=1)
                    nc.vector.reciprocal(rc[:], dsrc[:])
                    rb = fp_.tile([128, T], F32, tag=f"rb{slot}", name=f"rb{slot}", bufs=1)
                    nc.gpsimd.partition_broadcast(rb[:], rc[0:1, :])
                    recips.append(rb)
                of = []
                for slot in (0, 1):
                    o = fp_.tile([128, T], F32R, tag=f"of{slot}", name=f"of{slot}", bufs=1)
                    for t in range(NT):
                        in0 = out0_ps[t][:] if slot == 0 else \
                            out1_sb[:, t * TC:(t + 1) * TC]
                        nc.vector.tensor_tensor(
                            out=o[:, t * TC:(t + 1) * TC], in0=in0,
                            in1=recips[slot][:, t * TC:(t + 1) * TC],
                            op=ALU.mult)
                    of.append(o)
                for jc in range(NJ):
                    for t in range(NT):
                        yp = yps.tile([128, TC], F32, tag="yp")
                        nc.tensor.matmul(
                            yp[:], woT_sb[:, jc * 128:(jc + 1) * 128],
                            of[0][:, t * TC:(t + 1) * TC],
                            start=True, stop=False)
                        nc.tensor.matmul(
                            yp[:], woT_sb[:, XD + jc * 128:XD + (jc + 1) * 128],
                            of[1][:, t * TC:(t + 1) * TC],
                            start=False, stop=True)
                        ysb = fp_.tile([128, TC], F32, tag="ysb")
                        nc.scalar.activation(ysb[:], yp[:], AF.Identity,
                                             bias=bo8[:, jc:jc + 1])
                        nc.sync.dma_start(
                            yT_d.ap()[jc * 128:(jc + 1) * 128,
                                      t * TC:(t + 1) * TC], ysb[:])

    nc.compile()
    return nc


# ---------------- host side ----------------

def host_prepare(cfg, x, freqs_cos, freqs_sin, wq, bq, wk, bk, wv, bv,
                 wo, bo, gq, gk, win_old_k, win_old_v, n_cores=8):
    """win_old_k/v: [L0, XD] assembled old window (eviction applied)."""
    T, XD = cfg["T"], cfg["XD"]
    L0, L1 = cfg["L0"], cfg["L1"]
    NEW = T
    split = L1 - NEW
    assert 0 <= split <= L0 and L0 - split <= L1, (L0, L1, split)
    n_sub1 = len(subchunks(L1))
    n_new = len(subchunks(NEW))

    f32 = np.float32
    xT = np.ascontiguousarray(x.reshape(T, XD).T, f32)
    cos2 = np.ascontiguousarray(np.concatenate([freqs_cos.T, freqs_cos.T], 0), f32)
    sin2 = np.ascontiguousarray(np.concatenate([freqs_sin.T, freqs_sin.T], 0), f32)

    in_maps = []
    for c in range(n_cores):
        p, role = divmod(c, 2)
        h0 = 3 * p + (0 if role == 0 else 2)
        h1 = 3 * p + 1
        cols = np.r_[h0 * 128:(h0 + 1) * 128, h1 * 128:(h1 + 1) * 128]
        cols0 = np.r_[h0 * 128:(h0 + 1) * 128]
        cols1 = np.r_[h1 * 128:(h1 + 1) * 128]

        m = {"xT": xT, "cos2": cos2, "sin2": sin2}
        m["wq"] = np.ascontiguousarray(wq[cols, :].T, f32)
        m["wk"] = np.ascontiguousarray(wk[cols, :].T, f32)
        m["wv"] = np.ascontiguousarray(wv[cols, :].T, f32)
        m["woT"] = np.ascontiguousarray(wo[:, cols].T, f32)
        m["bq"] = np.ascontiguousarray(bq[cols][:, None], f32)
        m["bk"] = np.ascontiguousarray(bk[cols][:, None], f32)
        m["bv"] = np.ascontiguousarray(bv[cols][:, None], f32)
        m["gq"] = np.ascontiguousarray(gq[cols][:, None], f32)
        m["gk"] = np.ascontiguousarray(gk[cols][:, None], f32)
        m["bo"] = np.ascontiguousarray(bo[:, None], f32)

        sqmask = np.zeros((256, 1), f32)
        sqmask[0:128] = 1.0
        q4 = 32
        if role == 0:
            sqmask[128:128 + q4] = 1.0
            sqmask[128 + 2 * q4:128 + 3 * q4] = 1.0
        else:
            sqmask[128 + q4:128 + 2 * q4] = 1.0
            sqmask[128 + 3 * q4:] = 1.0
        m["sqmask"] = sqmask
        m["identc"] = np.eye(128, dtype=f32)
        swpc = np.zeros((128, 128), f32)
        swpc[np.arange(64), np.arange(64) + 64] = 1.0
        swpc[np.arange(64) + 64, np.arange(64)] = -1.0
        m["swpc"] = swpc

        m["kc0T"] = np.ascontiguousarray(win_old_k[:, cols0].T, f32)
        m["vc0"] = np.ascontiguousarray(win_old_v[:, cols0], f32)

        k1 = np.zeros((L1, 128), f32)
        v1 = np.zeros((L1, 128), f32)
        if role == 0:
            k1[0:split] = win_old_k[0:split][:, cols1]
            v1[0:split] = win_old_v[0:split][:, cols1]
            valid1 = split
        else:
            valid1 = L0 - split
            k1[0:valid1] = win_old_k[split:L0][:, cols1]
            v1[0:valid1] = win_old_v[split:L0][:, cols1]
        m["kc1T"] = np.ascontiguousarray(k1.T)
        m["vc1"] = v1

        bias1 = np.zeros((128, n_sub1), f32)
        for j, (off, ck) in enumerate(subchunks(L1)):
            lv = int(np.clip(valid1 - off, 0, 128))
            bias1[lv:, j] = NEG_BIAS
        m["bias1"] = bias1
        bias2 = np.zeros((128, n_new), f32)
        if role == 1:
            bias2[:] = NEG_BIAS
        m["bias2"] = bias2
        in_maps.append(m)
    return in_maps


def host_finalize(cfg, results):
    y = None
    for r in results:
        y = r["yT"].astype(np.float64) if y is None \
            else y + r["yT"].astype(np.float64)
    return np.ascontiguousarray(y.T)[None].astype(np.float32)  # [1, T, XD]


def numpy_reference(cfg, x, freqs_cos, freqs_sin, wq, bq, wk, bk, wv, bv,
                    wo, bo, gq, gk, win_old_k, win_old_v):
    """Reference for arbitrary cfg: attention over [old window; new]."""
    T, XD, D = cfg["T"], cfg["XD"], cfg["D"]
    H = XD // D
    x2 = x.reshape(T, XD).astype(np.float64)

    def rms(t, g):
        return t / np.sqrt((t ** 2).mean(-1, keepdims=True) + EPS) * g

    q = rms(x2 @ wq.T + bq, gq)
    k = rms(x2 @ wk.T + bk, gk)
    v = x2 @ wv.T + bv

    def rope(t):
        th = t.reshape(T, H, D)
        t1, t2 = th[..., :D // 2], th[..., D // 2:]
        c = freqs_cos[:, None, :]
        s = freqs_sin[:, None, :]
        return np.concatenate([t1 * c - t2 * s, t1 * s + t2 * c],
                              -1).reshape(T, XD)

    rq, rk = rope(q), rope(k)
    kw = np.concatenate([win_old_k, rk], 0).reshape(-1, H, D)
    vw = np.concatenate([win_old_v, v], 0).reshape(-1, H, D)
    qh = rq.reshape(T, H, D)
    scores = np.einsum("thd,shd->hts", qh, kw) / math.sqrt(D)
    e = np.exp(scores - scores.max(-1, keepdims=True))
    probs = e / e.sum(-1, keepdims=True)
    out = np.einsum("hts,shd->thd", probs, vw).reshape(T, XD)
    return (out @ wo.T + bo)[None].astype(np.float32)


# =====================================================================
# kernel() entry point — full inputs in, full output out.
# =====================================================================

import os as _os
import time as _time
from concourse import bass_utils as _bass_utils

_DIM = 1536
_HEADS = 12
_HD = 128
_FRAME = 1560
_LOCAL_ATTN_SIZE = 6
_SINK_SIZE = 1
_CACHE = _LOCAL_ATTN_SIZE * _FRAME
_SINK = _SINK_SIZE * _FRAME
_MAX_ATTN = _LOCAL_ATTN_SIZE * _FRAME
_GLOBAL_END = _CACHE
_LOCAL_END = _CACHE

_prog_cache = {}
last_exec_ns = None
last_wall_ns = None


def _window_index(current_start, T):
    """Mirrors the reference's rolling-cache index math; returns original
    cache row indices of the attention window's old part."""
    cur_end = current_start + T
    if cur_end > _GLOBAL_END and T + _LOCAL_END > _CACHE:
        evict = T + _LOCAL_END - _CACHE
        rolled = _LOCAL_END - evict - _SINK
        le = _LOCAL_END + cur_end - _GLOBAL_END - evict
    else:
        evict, rolled = 0, 0
        le = _LOCAL_END + cur_end - _GLOBAL_END
    ls = le - T
    ws = max(0, le - _MAX_ATTN)
    idx = np.arange(ws, ls)
    if evict:
        shift = (idx >= _SINK) & (idx < _SINK + rolled)
        idx = np.where(shift, idx + evict, idx)
    return idx


def kernel(**inputs):
    global last_exec_ns, last_wall_ns
    x = np.ascontiguousarray(np.asarray(inputs["x"], np.float32))
    B, T, XD = x.shape
    assert B == 1 and XD == _DIM
    cs = int(np.asarray(inputs["current_start"]))
    idx_old = _window_index(cs, T)
    L0 = len(idx_old)
    L1 = max(T, (L0 + T + 1) // 2)
    NT = 1
    for cand in (4, 3, 2):
        if T % cand == 0 and T // cand <= 512:
            NT = cand
            break
    if T <= 512:
        NT = 1
    cfg = dict(T=T, NT=NT, XD=XD, D=_HD, L0=L0, L1=L1, SUPER=512)

    key = tuple(sorted(cfg.items()))
    if key not in _prog_cache:
        _prog_cache[key] = build_program(cfg, n_cores=8)
    nc = _prog_cache[key]

    f32 = np.float32
    ck = np.asarray(inputs["cache_k"], f32)[0]
    cv = np.asarray(inputs["cache_v"], f32)[0]
    win_k = np.ascontiguousarray(ck[idx_old])
    win_v = np.ascontiguousarray(cv[idx_old])
    args = (x, np.asarray(inputs["freqs_cos"], f32),
            np.asarray(inputs["freqs_sin"], f32),
            np.asarray(inputs["wq"], f32), np.asarray(inputs["bq"], f32),
            np.asarray(inputs["wk"], f32), np.asarray(inputs["bk"], f32),
            np.asarray(inputs["wv"], f32), np.asarray(inputs["bv"], f32),
            np.asarray(inputs["wo"], f32), np.asarray(inputs["bo"], f32),
            np.asarray(inputs["gq"], f32), np.asarray(inputs["gk"], f32),
            win_k, win_v)
    in_maps = host_prepare(cfg, *args, n_cores=8)

    trace = bool(int(_os.environ.get("WAN_KERNEL_TRACE", "0")))
    t0 = _time.time()
    try:
        res = _bass_utils.run_bass_kernel_spmd(
            nc, in_maps, core_ids=list(range(8)), trace=trace)
    except (ImportError, ModuleNotFoundError):
        # NTFF profile hook unavailable on this client image
        res = _bass_utils.run_bass_kernel_spmd(
            nc, in_maps, core_ids=list(range(8)))
    last_wall_ns = int((_time.time() - t0) * 1e9)
    last_exec_ns = res.exec_time_ns
    return host_finalize(cfg, [res.results[c] for c in range(8)])



# revision 46
# speedup vs baseline: 2.3277x; 2.3277x over previous
"""CausalWanModel sparse-attention TRN2 kernel.

Sharding (8 cores, 12 heads of HD=128): pair p in {0,1,2,3} owns heads
{3p, 3p+1, 3p+2}; core 2p ("A") fully owns head 3p, core 2p+1 ("B") fully
owns head 3p+2 (slot 0); both share head 3p+1 (slot 1), split by window
position: A covers cache[0:L0/2] + new[0:T/2], B covers cache[L0/2:L0] +
new[T/2:T].  SPMD: identical program on all cores, per-core data and
exp-bias masks; partial outputs are summed on the host (wo distributes
over the head dim), so only two tiny collectives are needed.

Engine plan:
  PE   : all matmuls in bf16 (1 cyc/row) - scores, PV, projections, wo.
  Act  : softmax exp (maximal [128,1024]+[128,536] instrs), proj biases,
         squares for the q/k sum-of-squares.
  DVE  : denominator accumulation (bf16 2x, epoch-flushed to fp32),
         rope-q elementwise, normalize + output-projection evacuation.
  Pool : rope-k elementwise, broadcasts, slot-1 out evacuation (SBUF only;
         GPSIMD cannot touch PSUM).
  PSUM : 4 banks PV accumulators + 2x2 banks double-buffered score tiles.
  DMA  : rotate-half rope swap done as SBUF->SBUF partition-swap DMA.

Softmax skips max-subtraction (|scores| <= sqrt(128) after RMSNorm).
RMSNorm factors need a cross-core sum of squares: one combined AllGather
(q||k halves) + a ones-matmul; rsqrt via DVE fast reciprocal + Act sqrt.
Slot-1 denominators pair-AllReduce mid-attention, hidden behind slot 0.
The final y is per-core partial in bf16; host sums cores and adds bo.

Timeline-sim predicted ~379 us/core vs ~560 us for the f32r baseline.
"""

import math
import contextlib
import numpy as np

import concourse.bass as bass
import concourse.tile as tile
from concourse import bacc, mybir

F32 = mybir.dt.float32
F32R = mybir.dt.float32r
BF16 = mybir.dt.bfloat16
AF = mybir.ActivationFunctionType
ALU = mybir.AluOpType

EPS = 1e-6
NEG_BIAS = -60.0   # exp(x + NEG_BIAS) ~ 0 for masked lanes
EPOCH = 16         # chunks per bf16 partial-sum epoch


def ceil_div(a, b):
    return -(-a // b)


def qtiles(T):
    """Query tiling into <=512 psum-bank-sized tiles: [512,512,512,24] for 1560."""
    out = []
    off = 0
    while off < T:
        out.append((off, min(512, T - off)))
        off += 512
    return out


def qhalves(T):
    """Two exp-instruction spans per chunk: tiles [0,1] and tiles [2...]."""
    qt = qtiles(T)
    h0 = qt[:2]
    h1 = qt[2:]
    a = (h0[0][0], sum(w for _, w in h0), h0)
    b = (h1[0][0], sum(w for _, w in h1), h1)
    return [a, b]


def full_cfg():
    return dict(T=1560, XD=1536, D=128, L0=7800, SUPER=512)


def build_program(cfg, n_cores=8):
    T, XD, D = cfg["T"], cfg["XD"], cfg["D"]
    L0, SUPER = cfg["L0"], cfg["SUPER"]
    NEW = T
    NK = XD // 128
    NJ = NK
    inv_sqrt_d = 1.0 / math.sqrt(D)
    QT = qtiles(T)
    QH = qhalves(T)
    NQT = len(QT)

    # chunk schedules
    half0 = ceil_div(L0, 2)                      # A dram keys; B gets L0-half0
    n_d1 = ceil_div(max(half0, L0 - half0), 128)  # slot1 dram chunks (31)
    L1D = n_d1 * 128
    n_d0 = ceil_div(L0, 128)                     # slot0 dram chunks (61)
    L0D = n_d0 * 128
    n_s = ceil_div(NEW, 128)                     # sbuf chunks (13)
    KPAD = n_s * 128                             # padded new-key width (1664)
    nch = {1: n_d1 + n_s, 0: n_d0 + n_s}
    # bias column ids: slot1 dram last, slot1 sbuf all, slot0 dram last, slot0 sbuf last
    bias_cols = {}
    nb = 0
    for c in range(n_d1 + n_s):
        if c >= n_d1 - 1:     # last dram chunk + all sbuf chunks
            bias_cols[(1, c)] = nb
            nb += 1
    bias_cols[(0, n_d0 - 1)] = nb; nb += 1
    bias_cols[(0, n_d0 + n_s - 1)] = nb; nb += 1

    nc = bacc.Bacc("TRN2", target_bir_lowering=False, debug=False,
                   num_devices=n_cores)

    def din(name, shape, dt=BF16):
        return nc.dram_tensor(name, shape, dt, kind="ExternalInput")

    xT_d = din("xT", [XD, T])
    w_d = {nm: din("w" + nm, [XD, 256]) for nm in ("q", "k", "v")}
    woT_d = din("woT", [256, XD])
    b_d = {nm: din("b" + nm, [256, 1], F32) for nm in ("q", "k", "v")}
    cs_d = {(nm, cc, tr): din(f"cs_{nm}{cc}{tr}", [128, T], BF16)
            for nm in ("q", "k") for cc in range(2) for tr in ("c", "s")}
    sqmask_d = din("sqmask", [256, 1], BF16)
    kc0_d = din("kc0T", [128, L0D])
    vc0_d = din("vc0", [L0D, 128])
    kc1_d = din("kc1T", [128, L1D])
    vc1_d = din("vc1", [L1D, 128])
    ident_d = din("identb", [128, 128], BF16)
    ones_d = din("onesc", [128, 1], F32R)
    bias_d = din("biast", [128, nb], F32)
    yT_d = nc.dram_tensor("yT", [XD, T], BF16, kind="ExternalOutput")

    with tile.TileContext(nc) as tc, contextlib.ExitStack() as ctx:
        const = ctx.enter_context(tc.tile_pool(name="const", bufs=1))
        persist = ctx.enter_context(tc.tile_pool(name="persist", bufs=1))
        dram = ctx.enter_context(tc.tile_pool(name="dram", bufs=1, space="DRAM"))

        ident = const.tile([128, 128], BF16)
        nc.sync.dma_start(ident[:], ident_d.ap())
        ones_r = const.tile([128, 1], F32R)
        nc.sync.dma_start(ones_r[:], ones_d.ap())
        ones8 = ones_r[0:8, :]
        eps_c = const.tile([1, 1], F32)
        nc.vector.memset(eps_c[:], EPS)
        biases = const.tile([128, 6], F32)
        for i, nm in enumerate(("q", "k", "v")):
            nc.sync.dma_start(biases[:, 2 * i:2 * i + 1], b_d[nm].ap()[0:128, :])
            nc.sync.dma_start(biases[:, 2 * i + 1:2 * i + 2],
                              b_d[nm].ap()[128:256, :])
        sqmask = const.tile([128, 2], BF16)
        nc.sync.dma_start(sqmask[:, 0:1], sqmask_d.ap()[0:128, :])
        nc.sync.dma_start(sqmask[:, 1:2], sqmask_d.ap()[128:256, :])
        biast = const.tile([128, nb], F32)
        nc.sync.dma_start(biast[:], bias_d.ap())

        cbuf = {}
        for nm, wid, nr in (("sq", 2 * T, n_cores), ("den1", T, 1)):
            cin = dram.tile([1, wid], F32, tag=f"cin_{nm}", name=f"cin_{nm}")
            cout = dram.tile([nr, wid], F32, tag=f"cout_{nm}",
                             name=f"cout_{nm}")
            cbuf[nm] = (cin, cout)
        groups_all = [list(range(n_cores))]
        groups_pair = [[i, i + 1] for i in range(0, n_cores, 2)]

        # persistent tiles
        qbf = [persist.tile([128, T], BF16, tag=f"qbf{s}", name=f"qbf{s}")
               for s in range(2)]
        kbf = [persist.tile([128, KPAD], BF16, tag=f"kbf{s}", name=f"kbf{s}")
               for s in range(2)]
        v_nat = [persist.tile([128, KPAD], BF16, tag=f"vn{s}", name=f"vn{s}")
                 for s in range(2)]
        acc_lo = persist.tile([128, T], BF16, tag="aclo")
        acc_hi = persist.tile([128, T], F32R, tag="achi")

        rms_b = {nm: persist.tile([128, T], F32, tag=f"rms{nm}", name=f"rms{nm}")
                 for nm in ("q", "k")}

        # zero padded tails once (avoid NaN poison via PV stationary garbage)
        for s in range(2):
            nc.gpsimd.memset(kbf[s][:], 0.0)
            nc.gpsimd.memset(v_nat[s][:], 0.0)


        # ============ P1: projections + sumsq + rope ============
        mid = ctx.enter_context(tc.tile_pool(name="mid", bufs=1))
        rq_sb = {nm: [mid.tile([128, T], BF16, tag=f"rq{nm}{cc}",
                               name=f"rq{nm}{cc}") for cc in range(2)]
                 for nm in ("q", "k")}
        xv = [mid.tile([128, T], BF16, tag=f"xv{cc}", name=f"xv{cc}")
              for cc in range(2)]
        shuf_sb = [mid.tile([128, T], F32, tag=f"shf{cc}", name=f"shf{cc}")
                   for cc in range(2)]
        xpp = ctx.enter_context(tc.tile_pool(name="xp", bufs=1))
        xp = {nm: [xpp.tile([128, T], F32, tag=f"x{nm}{cc}",
                            name=f"x{nm}{cc}") for cc in range(2)]
              for nm in ("q", "k")}
        rmsq_pool = ctx.enter_context(tc.tile_pool(name="rmsq", bufs=2))
        p1 = contextlib.ExitStack()
        xpool = p1.enter_context(tc.tile_pool(name="xT", bufs=1))
        wpool = p1.enter_context(tc.tile_pool(name="wstr", bufs=2))
        rpc = p1.enter_context(tc.tile_pool(name="ropec", bufs=1))
        m2_pool = p1.enter_context(tc.tile_pool(name="m2p", bufs=1))
        pps = p1.enter_context(tc.tile_pool(name="proj_ps", bufs=1,
                                            space="PSUM"))
        sps = p1.enter_context(tc.tile_pool(name="sq_ps", bufs=1,
                                            space="PSUM"))
        sqt_pool = p1.enter_context(tc.tile_pool(name="sqt", bufs=2))

        xT_sb = xpool.tile([128, NK * T], BF16)
        sq_all = xpool.tile([1, 2 * T], F32, tag="sqall")
        sq_sb = {"q": sq_all[:, 0:T], "k": sq_all[:, T:2 * T]}
        w_sb = {}
        cs_sb = {}

        def load_cs(nm):
            for cc in range(2):
                for tr in ("c", "s"):
                    t_ = rpc.tile([128, T], BF16, tag=f"cs{cc}{tr}",
                                  name=f"cs_{nm}{cc}{tr}")
                    nc.sync.dma_start(t_[:], cs_d[(nm, cc, tr)].ap())
                    cs_sb[(nm, cc, tr)] = t_

        XG = 3  # xT chunks per DMA descriptor

        def emit_proj(nm, ccs=(0, 1), bias_eng=None):
            ib = ("q", "k", "v").index(nm)
            if nm not in w_sb:
                w_sb[nm] = wpool.tile([128, NK * 256], BF16, tag="w",
                                      name=f"w{nm}", bufs=2)
                nc.sync.dma_start(
                    w_sb[nm][:].rearrange("p (k c) -> p k c", k=NK),
                    w_d[nm].ap().rearrange("(k p) c -> p k c", p=128))
            for cc in ccs:
                ps = [pps.tile([128, 512], F32, tag=f"proj{t}",
                               name=f"proj{t}") for t in range(NQT)]
                for kc in range(NK):
                    if nm == "q" and cc == 0 and kc % XG == 0:
                        g0, g1 = kc, min(NK, kc + XG)
                        nc.sync.dma_start(
                            xT_sb[:, g0 * T:g1 * T].rearrange(
                                "p (k t) -> p k t", k=g1 - g0),
                            xT_d.ap()[g0 * 128:g1 * 128, :].rearrange(
                                "(k p) t -> p k t", p=128))
                    wt = w_sb[nm][:, kc * 256 + cc * 128:
                                  kc * 256 + cc * 128 + 128]
                    for t, (qo, qw) in enumerate(QT):
                        nc.tensor.matmul(
                            ps[t][:, 0:qw], wt,
                            xT_sb[:, kc * T + qo:kc * T + qo + qw],
                            start=(kc == 0), stop=(kc == NK - 1))
                dst = xv[cc] if nm == "v" else xp[nm][cc]
                beng = bias_eng or nc.scalar
                for t, (qo, qw) in enumerate(QT):
                    bias_ap = biases[:, 2 * ib + cc:2 * ib + cc + 1]
                    if beng is nc.scalar:
                        nc.scalar.add(dst[:, qo:qo + qw], ps[t][:, 0:qw],
                                      bias_ap)
                    else:
                        beng.tensor_scalar_add(dst[:, qo:qo + qw],
                                               ps[t][:, 0:qw], bias_ap)

        sq_tiles = {}

        def emit_squares(nm):
            for cc in range(2):
                for qo, qw in QT:
                    sqt = sqt_pool.tile([128, 512], BF16,
                                        tag=f"sqt{cc}{qo}",
                                        name=f"sqt{cc}{qo}", bufs=1)
                    sq_tiles[(nm, cc, qo)] = sqt
                    nc.scalar.activation(sqt[:, 0:qw],
                                         xp[nm][cc][:, qo:qo + qw],
                                         AF.Square)

        def emit_sq_mm(nm):
            for qo, qw in QT:
                qps = sps.tile([1, 512], F32, tag="sqps")
                for cc in range(2):
                    sqt = sq_tiles[(nm, cc, qo)]
                    nc.tensor.matmul(qps[:, 0:qw], sqmask[:, cc:cc + 1],
                                     sqt[:, 0:qw], start=(cc == 0),
                                     stop=(cc == 1))
                nc.scalar.copy(sq_sb[nm][0:1, qo:qo + qw], qps[:, 0:qw])

        def emit_sq_launch():
            nc.scalar.dma_start(cbuf["sq"][0][:], sq_all[:])
            nc.gpsimd.collective_compute(
                "AllGather", ALU.bypass, replica_groups=groups_all,
                ins=[cbuf["sq"][0].opt()], outs=[cbuf["sq"][1].opt()])

        def emit_shuffles(nm):
            # rotate-half via SBUF->SBUF DMA (partition swap, no engine time)
            half = 64
            for cc in range(2):
                nc.sync.dma_start(shuf_sb[cc][0:half, :],
                                  xp[nm][cc][half:128, :])
                nc.sync.dma_start(shuf_sb[cc][half:128, :],
                                  xp[nm][cc][0:half, :])

        def emit_rope_rest(nm, eng):
            # m1 = x*cos_g; m2 = shuffled(x)*sin_signed_g; rq = m1+m2
            for cc in range(2):
                m1 = rq_sb[nm][cc]
                eng.tensor_tensor(out=m1[:], in0=xp[nm][cc][:],
                                  in1=cs_sb[(nm, cc, "c")][:], op=ALU.mult)
                m2 = m2_pool.tile([128, T], F32, tag="m2")
                eng.tensor_tensor(out=m2[:], in0=shuf_sb[cc][:],
                                  in1=cs_sb[(nm, cc, "s")][:], op=ALU.mult)
                eng.tensor_tensor(out=m1[:], in0=m1[:], in1=m2[:],
                                  op=ALU.add)

        def emit_rope_pre(nm, eng):
            emit_shuffles(nm)
            emit_rope_rest(nm, eng)

        def rms_finalize(nm):
            off = 0 if nm == "q" else T
            parts = rmsq_pool.tile([n_cores, T], F32R, tag="rparts",
                                   name=f"rparts{nm}", bufs=1)
            nc.gpsimd.dma_start(
                parts[:], cbuf["sq"][1][:, off:off + T])
            tot = rmsq_pool.tile([1, T], F32, tag="rtmp", name="rtot")
            ps_pool, ptag, pshape = rms_ps[nm]
            for qo, qw in QT:
                rp_ = ps_pool.tile(pshape, F32, tag=ptag)
                nc.tensor.matmul(rp_[0:1, 0:qw], ones8[:],
                                 parts[:, qo:qo + qw], start=True, stop=True)
                nc.vector.tensor_copy(tot[0:1, qo:qo + qw], rp_[0:1, 0:qw])
            srt = rmsq_pool.tile([1, T], F32, tag="rtmp", name="rsrt")
            nc.scalar.activation(srt[:], tot[:], AF.Sqrt,
                                 bias=eps_c[:], scale=1.0 / XD)
            rec = rmsq_pool.tile([1, T], F32, tag="rtmp", name="rrec")
            nc.vector.reciprocal_approx_fast(rec[:], srt[:])
            if nm == "q":
                rec2 = rmsq_pool.tile([1, T], F32, tag="rtmp", name="rrec2")
                nc.vector.tensor_scalar_mul(rec2[:], rec[:], inv_sqrt_d)
                rec = rec2
            nc.gpsimd.partition_broadcast(rms_b[nm][:], rec[0:1, :])
            meng = nc.vector if nm == "q" else nc.gpsimd
            for cc in range(2):
                dst = qbf[cc] if nm == "q" else kbf[cc]
                meng.tensor_tensor(out=dst[:, 0:T], in0=rq_sb[nm][cc][:],
                                   in1=rms_b[nm][:], op=ALU.mult)

        rms_ps = {"q": (sps, "sqps", [1, 512]),
                  "k": (sps, "sqps", [1, 512])}

        # --- emission schedule (engine-queue aware) ---
        emit_proj("q")
        load_cs("q")
        emit_squares("q")
        emit_proj("k")
        emit_sq_mm("q")
        emit_rope_pre("q", nc.vector)
        load_cs("k")
        emit_squares("k")
        emit_proj("v", ccs=(0,), bias_eng=nc.vector)
        emit_sq_mm("k")
        emit_sq_launch()
        emit_proj("v", ccs=(1,), bias_eng=nc.vector)
        emit_shuffles("k")
        rms_finalize("q")

        # --- v transpose to [keys, vd] (copies on DVE) ---
        for cc in range(2):
            for j2 in range(n_s):
                off = j2 * 128
                ck = min(128, T - off)
                tp = sps.tile([128, 128], BF16, tag="vtp")
                nc.tensor.matmul(tp[0:ck, :], xv[cc][:, off:off + ck],
                                 ident[:], is_transpose=True,
                                 start=True, stop=True)
                nc.vector.tensor_copy(
                    v_nat[cc][0:ck, j2 * 128:(j2 + 1) * 128], tp[0:ck, :])

        emit_rope_rest("k", nc.gpsimd)
        rms_finalize("k")

        # ============ P2: attention ============
        def attend(slot, n_dram, kd, vd, out_ps, stp, ep, hooks=None):
            nchunks = nch[slot]
            state = {"c": 0}

            def emit_tail(p):
                # deferred PV + denominator accumulate for chunk p["c"]
                c, ex, v_ap = p["c"], p["ex"], p["v"]
                first, last = c == 0, c == nchunks - 1
                for qo, qw, tiles in QH:
                    for o, w in tiles:
                        ti = QT.index((o, w))
                        nc.tensor.matmul(out_ps[ti][:, 0:w], v_ap,
                                         ex[:, o:o + w],
                                         start=first, stop=last,
                                         skip_group_check=True)
                if (c % EPOCH) == 0:
                    nc.vector.tensor_copy(acc_lo[:, 0:T], ex[:, 0:T])
                else:
                    nc.vector.tensor_tensor(out=acc_lo[:, 0:T],
                                            in0=acc_lo[:, 0:T],
                                            in1=ex[:, 0:T], op=ALU.add)
                # epoch flush (Pool mid-slot, DVE for the last: tail latency)
                if (c % EPOCH) == EPOCH - 1 or c == nchunks - 1:
                    feng = nc.vector if c == nchunks - 1 else nc.gpsimd
                    if c < EPOCH:
                        feng.tensor_copy(acc_hi[:], acc_lo[:])
                    else:
                        feng.tensor_tensor(out=acc_hi[:], in0=acc_hi[:],
                                           in1=acc_lo[:], op=ALU.add)
                if hooks and (c + 1) in hooks:
                    hooks.pop(c + 1)()

            def do_chunk(k_ap, v_ap, bias_ap):
                c = state["c"]
                sts = []
                for qo, qw, tiles in QH:
                    st = stp.tile([128, 1024], F32, tag="st")
                    for o, w in tiles:
                        nc.tensor.matmul(st[:, o - qo:o - qo + w], k_ap,
                                         qbf[slot][:, o:o + w],
                                         start=True, stop=True)
                    sts.append(st)
                ex = ep.tile([128, 2 * 1024], BF16, tag="ex")
                bias = 0.0 if bias_ap is None else bias_ap
                for (qo, qw, tiles), st in zip(QH, sts):
                    nc.scalar.activation(ex[:, qo:qo + qw], st[:, 0:qw],
                                         AF.Exp, bias=bias)
                if state.get("pend") is not None:
                    emit_tail(state["pend"])
                state["pend"] = {"c": c, "ex": ex, "v": v_ap}
                state["c"] += 1

            # dram segment, streamed in SUPER-key groups
            with tc.tile_pool(name=f"ks{slot}", bufs=3) as kp, \
                 tc.tile_pool(name=f"vs{slot}", bufs=3) as vp:
                LD = n_dram * 128
                soff = 0
                while soff < LD:
                    ssz = min(SUPER, LD - soff)
                    ks = kp.tile([128, SUPER], BF16, tag="ks")
                    nc.sync.dma_start(ks[:, 0:ssz], kd.ap()[:, soff:soff + ssz])
                    vs = vp.tile([128, SUPER], BF16, tag="vs")
                    nsub = ssz // 128
                    nc.sync.dma_start(
                        vs[:, 0:ssz].rearrange("p (j d) -> p j d", d=128),
                        vd.ap()[soff:soff + ssz, :].rearrange(
                            "(j p) d -> p j d", p=128))
                    for j in range(nsub):
                        c = state["c"]
                        bcol = bias_cols.get((slot, c))
                        bias_ap = None if bcol is None else biast[:, bcol:bcol + 1]
                        do_chunk(ks[:, j * 128:(j + 1) * 128],
                                 vs[:, j * 128:(j + 1) * 128], bias_ap)
                    soff += ssz
            # sbuf (new keys) segment
            for j in range(n_s):
                c = state["c"]
                bcol = bias_cols.get((slot, c))
                bias_ap = None if bcol is None else biast[:, bcol:bcol + 1]
                do_chunk(kbf[slot][:, j * 128:(j + 1) * 128],
                         v_nat[slot][:, j * 128:(j + 1) * 128], bias_ap)
            emit_tail(state["pend"])
            state["pend"] = None

        def den_reduce(slot, dps_pool):
            for gi in range(0, NQT, 2):
                dps = dps_pool.tile([128, 1024], F32, tag="st")
                for t in range(gi, min(gi + 2, NQT)):
                    qo, qw = QT[t]
                    o = (t - gi) * 512
                    nc.tensor.matmul(dps[0:1, o:o + qw], ones_r[:],
                                     acc_hi[:, qo:qo + qw], start=True,
                                     stop=True)
                    nc.vector.tensor_copy(den_sb[slot][0:1, qo:qo + qw],
                                          dps[0:1, o:o + qw])

        # ---- slot 1 ----
        p1.close()
        attn_sb = ctx.enter_context(tc.tile_pool(name="attn_sb", bufs=1))
        out1_sb = attn_sb.tile([128, T], BF16, tag="out1sb")
        den_sb = [attn_sb.tile([1, T], F32, tag=f"den{s2}", name=f"den{s2}")
                  for s2 in range(2)]
        of0 = attn_sb.tile([128, T], BF16, tag="of0")
        of1 = attn_sb.tile([128, T], BF16, tag="of1")
        with tc.tile_pool(name="ops1", bufs=1, space="PSUM") as ops1:
            out1_ps = [ops1.tile([128, w], F32, tag=f"o1_{t}", name=f"o1_{t}")
                       for t, (o, w) in enumerate(QT)]
            with tc.tile_pool(name="st1", bufs=2, space="PSUM") as stp1, \
                 tc.tile_pool(name="ex1", bufs=5) as ep1:
                attend(1, n_d1, kc1_d, vc1_d, out1_ps, stp1, ep1)
            # evacuate (unnormalized, bf16); den1 reduce deferred into slot 0
            for t, (qo, qw) in enumerate(QT):
                nc.vector.tensor_copy(out1_sb[:, qo:qo + qw],
                                      out1_ps[t][:, 0:qw])

        # ---- slot 0 ----
        with tc.tile_pool(name="finc", bufs=1) as fc_:
            woT_sb = fc_.tile([128, 2 * XD], BF16)
            nc.sync.dma_start(woT_sb[:, 0:XD], woT_d.ap()[0:128, :])
            nc.sync.dma_start(woT_sb[:, XD:2 * XD], woT_d.ap()[128:256, :])

            def emit_of1():
                den1_tot = fc_.tile([1, T], F32, tag="d1t")
                nc.sync.dma_start(den1_tot[:], cbuf["den1"][1][:])
                rc = fc_.tile([1, T], F32, tag="rc1")
                nc.vector.reciprocal_approx_fast(rc[:], den1_tot[:])
                rb = fc_.tile([128, T], F32, tag="rb1")
                nc.gpsimd.partition_broadcast(rb[:], rc[0:1, :])
                for t, (qo, qw) in enumerate(QT):
                    nc.gpsimd.tensor_tensor(
                        out=of1[:, qo:qo + qw], in0=out1_sb[:, qo:qo + qw],
                        in1=rb[:, qo:qo + qw], op=ALU.mult)

            trigger = max(0, nch[0] - 18)

            def emit_den1():
                den_reduce(1, stp0)
                nc.gpsimd.dma_start(cbuf["den1"][0][:], den_sb[1][:])
                nc.gpsimd.collective_compute(
                    "AllReduce", ALU.add, replica_groups=groups_pair,
                    ins=[cbuf["den1"][0].opt()], outs=[cbuf["den1"][1].opt()])

            with tc.tile_pool(name="ops0", bufs=1, space="PSUM") as ops0:
                out0_ps = [ops0.tile([128, w], F32, tag=f"o0_{t}",
                                     name=f"o0_{t}")
                           for t, (o, w) in enumerate(QT)]
                with tc.tile_pool(name="st0", bufs=2, space="PSUM") as stp0, \
                     tc.tile_pool(name="ex0", bufs=5) as ep0:
                    attend(0, n_d0, kc0_d, vc0_d, out0_ps, stp0, ep0,
                           hooks={2: emit_den1, trigger: emit_of1})
                    den_reduce(0, stp0)

                rc0 = fc_.tile([1, T], F32, tag="rc0")
                nc.vector.reciprocal_approx_fast(rc0[:], den_sb[0][:])
                rb0 = fc_.tile([128, T], F32, tag="rb0")
                nc.gpsimd.partition_broadcast(rb0[:], rc0[0:1, :])
                for t, (qo, qw) in enumerate(QT):
                    nc.vector.tensor_tensor(
                        out=of0[:, qo:qo + qw], in0=out0_ps[t][:, 0:qw],
                        in1=rb0[:, qo:qo + qw], op=ALU.mult)

            # ============ P3: output projection ============
            with tc.tile_pool(name="fin", bufs=3) as fp_, \
                 tc.tile_pool(name="yps", bufs=3, space="PSUM") as yps:
                u = 0
                for jc in range(NJ):
                    for hi, (qo, qw, tiles) in enumerate(QH):
                        yp = yps.tile([128, 1024], F32, tag="yp")
                        for o, w in tiles:
                            nc.tensor.matmul(
                                yp[:, o - qo:o - qo + w],
                                woT_sb[:, jc * 128:(jc + 1) * 128],
                                of0[:, o:o + w], start=True, stop=False)
                            nc.tensor.matmul(
                                yp[:, o - qo:o - qo + w],
                                woT_sb[:, XD + jc * 128:XD + (jc + 1) * 128],
                                of1[:, o:o + w], start=False, stop=True)
                        ysb = fp_.tile([128, 1024], BF16, tag="ysb")
                        u += 1
                        nc.vector.tensor_copy(ysb[:, 0:qw], yp[:, 0:qw])
                        nc.sync.dma_start(
                            yT_d.ap()[jc * 128:(jc + 1) * 128, qo:qo + qw],
                            ysb[:, 0:qw])

    nc.compile()
    return nc


# ---------------- host side ----------------

def host_prepare(cfg, x, freqs_cos, freqs_sin, wq, bq, wk, bk, wv, bv,
                 wo, bo, gq, gk, win_old_k, win_old_v, n_cores=8):
    """win_old_k/v: [L0, XD] assembled old window (eviction applied)."""
    import ml_dtypes
    bf16 = ml_dtypes.bfloat16
    T, XD = cfg["T"], cfg["XD"]
    L0 = cfg["L0"]
    NEW = T
    half0 = ceil_div(L0, 2)
    n_d1 = ceil_div(max(half0, L0 - half0), 128)
    L1D = n_d1 * 128
    n_d0 = ceil_div(L0, 128)
    L0D = n_d0 * 128
    n_s = ceil_div(NEW, 128)
    # bias layout must match build_program
    nb = (n_s + 1) + 2

    f32 = np.float32
    xT = np.ascontiguousarray(x.reshape(T, XD).T).astype(bf16)
    cos2 = np.concatenate([freqs_cos.T, freqs_cos.T], 0).astype(f32)
    sin2 = np.concatenate([freqs_sin.T, freqs_sin.T], 0).astype(f32)
    swap_idx = np.r_[64:128, 0:64]

    in_maps = []
    for c in range(n_cores):
        p, role = divmod(c, 2)
        h0 = 3 * p + (0 if role == 0 else 2)
        h1 = 3 * p + 1
        cols = np.r_[h0 * 128:(h0 + 1) * 128, h1 * 128:(h1 + 1) * 128]
        cols0 = np.r_[h0 * 128:(h0 + 1) * 128]
        cols1 = np.r_[h1 * 128:(h1 + 1) * 128]

        m = {"xT": xT}
        m["wq"] = np.ascontiguousarray(wq[cols, :].T).astype(bf16)
        m["wk"] = np.ascontiguousarray(wk[cols, :].T).astype(bf16)
        m["wv"] = np.ascontiguousarray(wv[cols, :].T).astype(bf16)
        m["woT"] = np.ascontiguousarray(wo[:, cols].T).astype(bf16)
        m["bq"] = np.ascontiguousarray(bq[cols][:, None], f32)
        m["bk"] = np.ascontiguousarray(bk[cols][:, None], f32)
        m["bv"] = np.ascontiguousarray(bv[cols][:, None], f32)
        # g folded into cos/sin tables per (nm, cc)
        for nm, g in (("q", gq), ("k", gk)):
            for cc in range(2):
                gc = g[cols[cc * 128:(cc + 1) * 128]]
                m[f"cs_{nm}{cc}c"] = np.ascontiguousarray(cos2 * gc[:, None]).astype(bf16)
                sgn = np.where(np.arange(128) < 64, -1.0, 1.0)[:, None]
                m[f"cs_{nm}{cc}s"] = np.ascontiguousarray(
                    sin2 * gc[swap_idx][:, None] * sgn).astype(bf16)

        sqmask = np.zeros((256, 1), f32)
        sqmask[0:128] = 1.0
        q4 = 32
        if role == 0:
            sqmask[128:128 + q4] = 1.0
            sqmask[128 + 2 * q4:128 + 3 * q4] = 1.0
        else:
            sqmask[128 + q4:128 + 2 * q4] = 1.0
            sqmask[128 + 3 * q4:] = 1.0
        m["sqmask"] = sqmask.astype(bf16)
        m["identb"] = np.eye(128).astype(bf16)
        m["onesc"] = np.ones((128, 1), f32)

        # slot0: full head h0 window
        k0 = np.zeros((128, L0D), bf16)
        k0[:, 0:L0] = win_old_k[:, cols0].T.astype(bf16)
        v0 = np.zeros((L0D, 128), bf16)
        v0[0:L0] = win_old_v[:, cols0].astype(bf16)
        m["kc0T"] = k0
        m["vc0"] = v0

        # slot1: half the cache for h1
        half0_ = half0
        if role == 0:
            lo, hi = 0, half0_
        else:
            lo, hi = half0_, L0
        nvalid1 = hi - lo
        k1 = np.zeros((128, L1D), bf16)
        k1[:, 0:nvalid1] = win_old_k[lo:hi][:, cols1].T.astype(bf16)
        v1 = np.zeros((L1D, 128), bf16)
        v1[0:nvalid1] = win_old_v[lo:hi][:, cols1].astype(bf16)
        m["kc1T"] = k1
        m["vc1"] = v1

        # bias table
        bias = np.zeros((128, nb), f32)
        col = 0
        # slot1 dram last chunk: valid keys in [ (n_d1-1)*128, nvalid1 )
        lv = int(np.clip(nvalid1 - (n_d1 - 1) * 128, 0, 128))
        bias[lv:, col] = NEG_BIAS
        col += 1
        # slot1 sbuf chunks: this core's new-token half
        nlo = 0 if role == 0 else NEW // 2
        nhi = NEW // 2 if role == 0 else NEW
        for j in range(n_s):
            base = j * 128
            valid = np.zeros(128, bool)
            for r in range(128):
                t_ = base + r
                valid[r] = (nlo <= t_ < nhi) and t_ < NEW
            bias[~valid, col] = NEG_BIAS
            col += 1
        # slot0 dram last
        lv = int(np.clip(L0 - (n_d0 - 1) * 128, 0, 128))
        bias[lv:, col] = NEG_BIAS
        col += 1
        # slot0 sbuf last
        lv = int(np.clip(NEW - (n_s - 1) * 128, 0, 128))
        bias[lv:, col] = NEG_BIAS
        col += 1
        m["biast"] = bias
        in_maps.append(m)
    return in_maps


def host_finalize(cfg, results, bo):
    y = None
    for r in results:
        yi = r["yT"].astype(np.float32)
        y = yi if y is None else y + yi
    y = y.T + bo[None, :]
    return np.ascontiguousarray(y)[None].astype(np.float32)  # [1, T, XD]


def numpy_reference(cfg, x, freqs_cos, freqs_sin, wq, bq, wk, bk, wv, bv,
                    wo, bo, gq, gk, win_old_k, win_old_v):
    """Reference for arbitrary cfg: attention over [old window; new]."""
    T, XD, D = cfg["T"], cfg["XD"], cfg["D"]
    H = XD // D
    x2 = x.reshape(T, XD).astype(np.float64)

    def rms(t, g):
        return t / np.sqrt((t ** 2).mean(-1, keepdims=True) + EPS) * g

    q = rms(x2 @ wq.T + bq, gq)
    k = rms(x2 @ wk.T + bk, gk)
    v = x2 @ wv.T + bv

    def rope(t):
        th = t.reshape(T, H, D)
        t1, t2 = th[..., :D // 2], th[..., D // 2:]
        c = freqs_cos[:, None, :]
        s = freqs_sin[:, None, :]
        return np.concatenate([t1 * c - t2 * s, t1 * s + t2 * c],
                              -1).reshape(T, XD)

    rq, rk = rope(q), rope(k)
    kw = np.concatenate([win_old_k, rk], 0).reshape(-1, H, D)
    vw = np.concatenate([win_old_v, v], 0).reshape(-1, H, D)
    qh = rq.reshape(T, H, D)
    scores = np.einsum("thd,shd->hts", qh, kw) / math.sqrt(D)
    e = np.exp(scores - scores.max(-1, keepdims=True))
    probs = e / e.sum(-1, keepdims=True)
    out = np.einsum("hts,shd->thd", probs, vw).reshape(T, XD)
    return (out @ wo.T + bo)[None].astype(np.float32)


# =====================================================================
# kernel() entry point — full inputs in, full output out.
# =====================================================================

import os as _os
import time as _time
from concourse import bass_utils as _bass_utils

_DIM = 1536
_HEADS = 12
_HD = 128
_FRAME = 1560
_LOCAL_ATTN_SIZE = 6
_SINK_SIZE = 1
_CACHE = _LOCAL_ATTN_SIZE * _FRAME
_SINK = _SINK_SIZE * _FRAME
_MAX_ATTN = _LOCAL_ATTN_SIZE * _FRAME
_GLOBAL_END = _CACHE
_LOCAL_END = _CACHE

_prog_cache = {}
last_exec_ns = None
last_wall_ns = None


def _window_index(current_start, T):
    """Mirrors the reference's rolling-cache index math; returns original
    cache row indices of the attention window's old part."""
    cur_end = current_start + T
    if cur_end > _GLOBAL_END and T + _LOCAL_END > _CACHE:
        evict = T + _LOCAL_END - _CACHE
        rolled = _LOCAL_END - evict - _SINK
        le = _LOCAL_END + cur_end - _GLOBAL_END - evict
    else:
        evict, rolled = 0, 0
        le = _LOCAL_END + cur_end - _GLOBAL_END
    ls = le - T
    ws = max(0, le - _MAX_ATTN)
    idx = np.arange(ws, ls)
    if evict:
        shift = (idx >= _SINK) & (idx < _SINK + rolled)
        idx = np.where(shift, idx + evict, idx)
    return idx


def kernel(**inputs):
    global last_exec_ns, last_wall_ns
    x = np.ascontiguousarray(np.asarray(inputs["x"], np.float32))
    B, T, XD = x.shape
    assert B == 1 and XD == _DIM
    cs = int(np.asarray(inputs["current_start"]))
    idx_old = _window_index(cs, T)
    L0 = len(idx_old)
    cfg = dict(T=T, XD=XD, D=_HD, L0=L0, SUPER=512)

    key = tuple(sorted(cfg.items()))
    if key not in _prog_cache:
        _prog_cache[key] = build_program(cfg, n_cores=8)
    nc = _prog_cache[key]

    f32 = np.float32
    ck = np.asarray(inputs["cache_k"], f32)[0]
    cv = np.asarray(inputs["cache_v"], f32)[0]
    win_k = np.ascontiguousarray(ck[idx_old])
    win_v = np.ascontiguousarray(cv[idx_old])
    bo = np.asarray(inputs["bo"], f32)
    args = (x, np.asarray(inputs["freqs_cos"], f32),
            np.asarray(inputs["freqs_sin"], f32),
            np.asarray(inputs["wq"], f32), np.asarray(inputs["bq"], f32),
            np.asarray(inputs["wk"], f32), np.asarray(inputs["bk"], f32),
            np.asarray(inputs["wv"], f32), np.asarray(inputs["bv"], f32),
            np.asarray(inputs["wo"], f32), bo,
            np.asarray(inputs["gq"], f32), np.asarray(inputs["gk"], f32),
            win_k, win_v)
    in_maps = host_prepare(cfg, *args, n_cores=8)

    trace = bool(int(_os.environ.get("WAN_KERNEL_TRACE", "0")))
    t0 = _time.time()
    try:
        res = _bass_utils.run_bass_kernel_spmd(
            nc, in_maps, core_ids=list(range(8)), trace=trace)
    except (ImportError, ModuleNotFoundError):
        res = _bass_utils.run_bass_kernel_spmd(
            nc, in_maps, core_ids=list(range(8)))
    last_wall_ns = int((_time.time() - t0) * 1e9)
    last_exec_ns = res.exec_time_ns
    return host_finalize(cfg, [res.results[c] for c in range(8)], bo)


# revision 49
# speedup vs baseline: 2.4582x; 1.0560x over previous
"""CausalWanModel sparse-attention TRN2 kernel.

Sharding (8 cores, 12 heads of HD=128): pair p in {0,1,2,3} owns heads
{3p, 3p+1, 3p+2}; core 2p ("A") fully owns head 3p, core 2p+1 ("B") fully
owns head 3p+2 (slot 0); both share head 3p+1 (slot 1), split by window
position: A covers cache[0:L0/2] + new[0:T/2], B covers cache[L0/2:L0] +
new[T/2:T].  SPMD: identical program on all cores, per-core data and
exp-bias masks; partial outputs are summed on the host (wo distributes
over the head dim), so only two tiny collectives are needed.

Engine plan:
  PE   : all matmuls in bf16 (1 cyc/row) - scores, PV, projections, wo.
  Act  : softmax exp (maximal [128,1024]+[128,536] instrs), proj biases,
         squares for the q/k sum-of-squares.
  DVE  : denominator accumulation (bf16 2x, epoch-flushed to fp32),
         rope-q elementwise, normalize + output-projection evacuation.
  Pool : rope-k elementwise, broadcasts, slot-1 out evacuation (SBUF only;
         GPSIMD cannot touch PSUM).
  PSUM : 4 banks PV accumulators + 2x2 banks double-buffered score tiles.
  DMA  : rotate-half rope swap done as SBUF->SBUF partition-swap DMA.

Softmax skips max-subtraction (|scores| <= sqrt(128) after RMSNorm).
RMSNorm factors need a cross-core sum of squares: one combined AllGather
(q||k halves) + a ones-matmul; rsqrt via DVE fast reciprocal + Act sqrt.
Slot-1 denominators pair-AllReduce mid-attention, hidden behind slot 0.
The final y is per-core partial in bf16; host sums cores and adds bo.

Timeline-sim predicted ~379 us/core vs ~560 us for the f32r baseline.
"""

import math
import contextlib
import numpy as np

import concourse.bass as bass
import concourse.tile as tile
from concourse import bacc, mybir

F32 = mybir.dt.float32
F32R = mybir.dt.float32r
BF16 = mybir.dt.bfloat16
AF = mybir.ActivationFunctionType
ALU = mybir.AluOpType

EPS = 1e-6
NEG_BIAS = -60.0   # exp(x + NEG_BIAS) ~ 0 for masked lanes
EPOCH = 16         # chunks per bf16 partial-sum epoch


def ceil_div(a, b):
    return -(-a // b)


def qtiles(T):
    """Query tiling into <=512 psum-bank-sized tiles: [512,512,512,24] for 1560."""
    out = []
    off = 0
    while off < T:
        out.append((off, min(512, T - off)))
        off += 512
    return out


def qhalves(T):
    """Two exp-instruction spans per chunk: tiles [0,1] and tiles [2...]."""
    qt = qtiles(T)
    h0 = qt[:2]
    h1 = qt[2:]
    a = (h0[0][0], sum(w for _, w in h0), h0)
    b = (h1[0][0], sum(w for _, w in h1), h1)
    return [a, b]


def full_cfg():
    return dict(T=1560, XD=1536, D=128, L0=7800, SUPER=512)


def build_program(cfg, n_cores=8):
    T, XD, D = cfg["T"], cfg["XD"], cfg["D"]
    L0, SUPER = cfg["L0"], cfg["SUPER"]
    NEW = T
    NK = XD // 128
    NJ = NK
    inv_sqrt_d = 1.0 / math.sqrt(D)
    QT = qtiles(T)
    QH = qhalves(T)
    NQT = len(QT)

    # chunk schedules
    half0 = ceil_div(L0, 2)                      # A dram keys; B gets L0-half0
    n_d1 = ceil_div(max(half0, L0 - half0), 128)  # slot1 dram chunks (31)
    L1D = n_d1 * 128
    n_d0 = ceil_div(L0, 128)                     # slot0 dram chunks (61)
    L0D = n_d0 * 128
    n_s = ceil_div(NEW, 128)                     # sbuf chunks (13)
    KPAD = n_s * 128                             # padded new-key width (1664)
    nch = {1: n_d1 + n_s, 0: n_d0 + n_s}
    # bias column ids: slot1 dram last, slot1 sbuf all, slot0 dram last, slot0 sbuf last
    bias_cols = {}
    nb = 0
    for c in range(n_d1 + n_s):
        if c >= n_d1 - 1:     # last dram chunk + all sbuf chunks
            bias_cols[(1, c)] = nb
            nb += 1
    bias_cols[(0, n_d0 - 1)] = nb; nb += 1
    bias_cols[(0, n_d0 + n_s - 1)] = nb; nb += 1

    nc = bacc.Bacc("TRN2", target_bir_lowering=False, debug=False,
                   num_devices=n_cores)

    def din(name, shape, dt=BF16):
        return nc.dram_tensor(name, shape, dt, kind="ExternalInput")

    xT_d = din("xT", [XD, T])
    w_d = {nm: din("w" + nm, [XD, 256]) for nm in ("q", "k", "v")}
    woT_d = din("woT", [256, XD])
    b_d = {nm: din("b" + nm, [256, 1], F32) for nm in ("q", "k", "v")}
    cs_d = {(nm, cc, tr): din(f"cs_{nm}{cc}{tr}", [128, T], BF16)
            for nm in ("q", "k") for cc in range(2) for tr in ("c", "s")}
    sqmask_d = din("sqmask", [256, 1], BF16)
    kc0_d = din("kc0T", [128, L0D])
    vc0_d = din("vc0", [L0D, 128])
    kc1_d = din("kc1T", [128, L1D])
    vc1_d = din("vc1", [L1D, 128])
    ident_d = din("identb", [128, 128], BF16)
    ones_d = din("onesc", [128, 1], F32R)
    onesrow_d = din("onesrow", [1, 128], F32R)
    bias_d = din("biast", [128, nb], F32)
    yT_d = nc.dram_tensor("yT", [XD, T], BF16, kind="ExternalOutput")

    with tile.TileContext(nc) as tc, contextlib.ExitStack() as ctx:
        const = ctx.enter_context(tc.tile_pool(name="const", bufs=1))
        persist = ctx.enter_context(tc.tile_pool(name="persist", bufs=1))
        dram = ctx.enter_context(tc.tile_pool(name="dram", bufs=1, space="DRAM"))

        ident = const.tile([128, 128], BF16)
        nc.sync.dma_start(ident[:], ident_d.ap())
        ones_r = const.tile([128, 1], F32R)
        nc.sync.dma_start(ones_r[:], ones_d.ap())
        ones8 = ones_r[0:8, :]
        ones_row = const.tile([1, 128], F32R)
        nc.sync.dma_start(ones_row[:], onesrow_d.ap())
        eps_c = const.tile([1, 1], F32)
        nc.vector.memset(eps_c[:], EPS)
        eps_cq = const.tile([1, 1], F32)
        nc.vector.memset(eps_cq[:], D * EPS)
        biases = const.tile([128, 6], F32)
        for i, nm in enumerate(("q", "k", "v")):
            nc.sync.dma_start(biases[:, 2 * i:2 * i + 1], b_d[nm].ap()[0:128, :])
            nc.sync.dma_start(biases[:, 2 * i + 1:2 * i + 2],
                              b_d[nm].ap()[128:256, :])
        sqmask = const.tile([128, 2], BF16)
        nc.sync.dma_start(sqmask[:, 0:1], sqmask_d.ap()[0:128, :])
        nc.sync.dma_start(sqmask[:, 1:2], sqmask_d.ap()[128:256, :])
        biast = const.tile([128, nb], F32)
        nc.sync.dma_start(biast[:], bias_d.ap())

        cbuf = {}
        for nm, wid, nr in (("sq", 2 * T, n_cores), ("den1", T, 1)):
            cin = dram.tile([1, wid], F32, tag=f"cin_{nm}", name=f"cin_{nm}")
            cout = dram.tile([nr, wid], F32, tag=f"cout_{nm}",
                             name=f"cout_{nm}")
            cbuf[nm] = (cin, cout)
        groups_all = [list(range(n_cores))]
        groups_pair = [[i, i + 1] for i in range(0, n_cores, 2)]

        # persistent tiles
        qbf = [persist.tile([128, T], BF16, tag=f"qbf{s}", name=f"qbf{s}")
               for s in range(2)]
        kbf = [persist.tile([128, KPAD], BF16, tag=f"kbf{s}", name=f"kbf{s}")
               for s in range(2)]
        v_nat = [persist.tile([128, KPAD], BF16, tag=f"vn{s}", name=f"vn{s}")
                 for s in range(2)]
        acc_lo = persist.tile([128, T], BF16, tag="aclo")
        acc_hi = persist.tile([128, T], F32R, tag="achi")


        # zero padded tails once (avoid NaN poison via PV stationary garbage)
        for s in range(2):
            nc.gpsimd.memset(kbf[s][:], 0.0)
            nc.gpsimd.memset(v_nat[s][:], 0.0)


        # ============ P1: projections + sumsq + rope ============
        mid = ctx.enter_context(tc.tile_pool(name="mid", bufs=1))
        rq_sb = {nm: [mid.tile([128, T], BF16, tag=f"rq{nm}{cc}",
                               name=f"rq{nm}{cc}") for cc in range(2)]
                 for nm in ("q", "k")}
        xv = [mid.tile([128, T], BF16, tag=f"xv{cc}", name=f"xv{cc}")
              for cc in range(2)]
        shuf_sb = [mid.tile([128, T], F32, tag=f"shf{cc}", name=f"shf{cc}")
                   for cc in range(2)]
        xpp = ctx.enter_context(tc.tile_pool(name="xp", bufs=1))
        xp = {nm: [xpp.tile([128, T], F32, tag=f"x{nm}{cc}",
                            name=f"x{nm}{cc}") for cc in range(2)]
              for nm in ("q", "k")}
        rmsq_pool = ctx.enter_context(tc.tile_pool(name="rmsq", bufs=2))
        p1 = contextlib.ExitStack()
        xpool = p1.enter_context(tc.tile_pool(name="xT", bufs=1))
        wpool = p1.enter_context(tc.tile_pool(name="wstr", bufs=2))
        rpc = p1.enter_context(tc.tile_pool(name="ropec", bufs=1))
        m2_pool = p1.enter_context(tc.tile_pool(name="m2p", bufs=1))
        pps = p1.enter_context(tc.tile_pool(name="proj_ps", bufs=1,
                                            space="PSUM"))
        sps = p1.enter_context(tc.tile_pool(name="sq_ps", bufs=1,
                                            space="PSUM"))
        sqt_pool = p1.enter_context(tc.tile_pool(name="sqt", bufs=2))

        xT_sb = xpool.tile([128, NK * T], BF16)
        sq_all = xpool.tile([1, 2 * T], F32, tag="sqall")
        sq_sb = {"q": sq_all[:, 0:T], "k": sq_all[:, T:2 * T]}
        w_sb = {}
        cs_sb = {}

        def load_cs(nm):
            for cc in range(2):
                for tr in ("c", "s"):
                    t_ = rpc.tile([128, T], BF16, tag=f"cs{cc}{tr}",
                                  name=f"cs_{nm}{cc}{tr}")
                    nc.sync.dma_start(t_[:], cs_d[(nm, cc, tr)].ap())
                    cs_sb[(nm, cc, tr)] = t_

        XG = 3  # xT chunks per DMA descriptor

        def emit_proj(nm, ccs=(0, 1), bias_eng=None):
            ib = ("q", "k", "v").index(nm)
            if nm not in w_sb:
                w_sb[nm] = wpool.tile([128, NK * 256], BF16, tag="w",
                                      name=f"w{nm}", bufs=2)
                nc.sync.dma_start(
                    w_sb[nm][:].rearrange("p (k c) -> p k c", k=NK),
                    w_d[nm].ap().rearrange("(k p) c -> p k c", p=128))
            for cc in ccs:
                ps = [pps.tile([128, 512], F32, tag=f"proj{t}",
                               name=f"proj{t}") for t in range(NQT)]
                for kc in range(NK):
                    if nm == "q" and cc == 0 and kc % XG == 0:
                        g0, g1 = kc, min(NK, kc + XG)
                        nc.sync.dma_start(
                            xT_sb[:, g0 * T:g1 * T].rearrange(
                                "p (k t) -> p k t", k=g1 - g0),
                            xT_d.ap()[g0 * 128:g1 * 128, :].rearrange(
                                "(k p) t -> p k t", p=128))
                    wt = w_sb[nm][:, kc * 256 + cc * 128:
                                  kc * 256 + cc * 128 + 128]
                    for t, (qo, qw) in enumerate(QT):
                        nc.tensor.matmul(
                            ps[t][:, 0:qw], wt,
                            xT_sb[:, kc * T + qo:kc * T + qo + qw],
                            start=(kc == 0), stop=(kc == NK - 1))
                dst = xv[cc] if nm == "v" else xp[nm][cc]
                beng = bias_eng or nc.vector
                for t, (qo, qw) in enumerate(QT):
                    bias_ap = biases[:, 2 * ib + cc:2 * ib + cc + 1]
                    if beng is nc.scalar:
                        nc.scalar.add(dst[:, qo:qo + qw], ps[t][:, 0:qw],
                                      bias_ap)
                    else:
                        beng.tensor_scalar_add(dst[:, qo:qo + qw],
                                               ps[t][:, 0:qw], bias_ap)

        sq_tiles = {}

        def emit_squares(nm):
            for cc in range(2):
                for qo, qw in QT:
                    sqt = sqt_pool.tile([128, 512], BF16,
                                        tag=f"sqt{cc}{qo}",
                                        name=f"sqt{cc}{qo}", bufs=1)
                    sq_tiles[(nm, cc, qo)] = sqt
                    nc.scalar.activation(sqt[:, 0:qw],
                                         xp[nm][cc][:, qo:qo + qw],
                                         AF.Square)

        def emit_sq_mm(nm):
            for qo, qw in QT:
                qps = sps.tile([1, 512], F32, tag="sqps")
                for cc in range(2):
                    sqt = sq_tiles[(nm, cc, qo)]
                    nc.tensor.matmul(qps[:, 0:qw], sqmask[:, cc:cc + 1],
                                     sqt[:, 0:qw], start=(cc == 0),
                                     stop=(cc == 1))
                nc.scalar.copy(sq_sb[nm][0:1, qo:qo + qw], qps[:, 0:qw])

        def emit_sq_launch():
            nc.scalar.dma_start(cbuf["sq"][0][:], sq_all[:])
            nc.gpsimd.collective_compute(
                "AllGather", ALU.bypass, replica_groups=groups_all,
                ins=[cbuf["sq"][0].opt()], outs=[cbuf["sq"][1].opt()])

        def emit_shuffles(nm):
            # rotate-half via SBUF->SBUF DMA (partition swap, no engine time)
            half = 64
            for cc in range(2):
                nc.sync.dma_start(shuf_sb[cc][0:half, :],
                                  xp[nm][cc][half:128, :])
                nc.sync.dma_start(shuf_sb[cc][half:128, :],
                                  xp[nm][cc][0:half, :])

        def emit_rope_rest(nm, eng):
            # m1 = x*cos_g; m2 = shuffled(x)*sin_signed_g; rq = m1+m2
            for cc in range(2):
                m1 = rq_sb[nm][cc]
                eng.tensor_tensor(out=m1[:], in0=xp[nm][cc][:],
                                  in1=cs_sb[(nm, cc, "c")][:], op=ALU.mult)
                m2 = m2_pool.tile([128, T], F32, tag="m2")
                eng.tensor_tensor(out=m2[:], in0=shuf_sb[cc][:],
                                  in1=cs_sb[(nm, cc, "s")][:], op=ALU.mult)
                eng.tensor_tensor(out=m1[:], in0=m1[:], in1=m2[:],
                                  op=ALU.add)

        def emit_rope_pre(nm, eng):
            emit_shuffles(nm)
            emit_rope_rest(nm, eng)

        def rms_finalize(nm):
            off = 0 if nm == "q" else T
            parts = rmsq_pool.tile([n_cores, T], F32R, tag="rparts",
                                   name=f"rparts{nm}", bufs=1)
            nc.gpsimd.dma_start(
                parts[:], cbuf["sq"][1][:, off:off + T])
            srt = rmsq_pool.tile([1, T], F32, tag="rtmp", name="rsrt")
            scale = (D * 1.0 / XD) if nm == "q" else (1.0 / XD)
            epsc = eps_cq if nm == "q" else eps_c
            for qo, qw in QT:
                rp_ = sps.tile([1, 512], F32, tag="sqps")
                nc.tensor.matmul(rp_[0:1, 0:qw], ones8[:],
                                 parts[:, qo:qo + qw], start=True, stop=True)
                nc.scalar.activation(srt[0:1, qo:qo + qw], rp_[0:1, 0:qw],
                                     AF.Sqrt, bias=epsc[:], scale=scale)
            rec = rmsq_pool.tile([1, T], F32, tag="rtmp", name="rrec")
            nc.vector.reciprocal_approx_fast(rec[:], srt[:])
            rec_r = rmsq_pool.tile([1, T], F32R, tag="rtmp2", name="rrecr")
            nc.vector.tensor_copy(rec_r[:], rec[:])
            for qo, qw in QT:
                rbc = sps.tile([128, 512], F32, tag="rbc", bufs=2)
                nc.tensor.matmul(rbc[:, 0:qw], ones_row[:],
                                 rec_r[0:1, qo:qo + qw],
                                 start=True, stop=True)
                for cc in range(2):
                    dst = qbf[cc] if nm == "q" else kbf[cc]
                    nc.vector.tensor_tensor(
                        out=dst[:, qo:qo + qw],
                        in0=rq_sb[nm][cc][:, qo:qo + qw],
                        in1=rbc[:, 0:qw], op=ALU.mult)

        # --- emission schedule (engine-queue aware) ---
        emit_proj("q")
        load_cs("q")
        emit_squares("q")
        emit_proj("k")
        emit_sq_mm("q")
        emit_rope_pre("q", nc.vector)
        load_cs("k")
        emit_squares("k")
        emit_proj("v", ccs=(0,), bias_eng=nc.vector)
        emit_sq_mm("k")
        emit_sq_launch()
        emit_proj("v", ccs=(1,), bias_eng=nc.vector)
        emit_shuffles("k")
        rms_finalize("q")

        # --- v transpose to [keys, vd] (copies on DVE) ---
        for cc in range(2):
            for j2 in range(n_s):
                off = j2 * 128
                ck = min(128, T - off)
                tp = sps.tile([128, 128], BF16, tag="vtp")
                nc.tensor.matmul(tp[0:ck, :], xv[cc][:, off:off + ck],
                                 ident[:], is_transpose=True,
                                 start=True, stop=True)
                nc.vector.tensor_copy(
                    v_nat[cc][0:ck, j2 * 128:(j2 + 1) * 128], tp[0:ck, :])

        emit_rope_rest("k", nc.gpsimd)
        rms_finalize("k")

        # ============ P2: attention ============
        def attend(slot, n_dram, kd, vd, out_ps, stp, ep, hooks=None):
            nchunks = nch[slot]
            state = {"c": 0}

            def emit_tail(p):
                # deferred PV + denominator accumulate for chunk p["c"]
                c, ex, v_ap = p["c"], p["ex"], p["v"]
                first, last = c == 0, c == nchunks - 1
                for qo, qw, tiles in QH:
                    for o, w in tiles:
                        ti = QT.index((o, w))
                        nc.tensor.matmul(out_ps[ti][:, 0:w], v_ap,
                                         ex[:, o:o + w],
                                         start=first, stop=last,
                                         skip_group_check=True)
                if (c % EPOCH) == 0:
                    nc.vector.tensor_copy(acc_lo[:, 0:T], ex[:, 0:T])
                else:
                    nc.vector.tensor_tensor(out=acc_lo[:, 0:T],
                                            in0=acc_lo[:, 0:T],
                                            in1=ex[:, 0:T], op=ALU.add)
                # epoch flush (Pool mid-slot, DVE for the last: tail latency)
                if (c % EPOCH) == EPOCH - 1 or c == nchunks - 1:
                    feng = nc.vector if c == nchunks - 1 else nc.gpsimd
                    if c < EPOCH:
                        feng.tensor_copy(acc_hi[:], acc_lo[:])
                    else:
                        feng.tensor_tensor(out=acc_hi[:], in0=acc_hi[:],
                                           in1=acc_lo[:], op=ALU.add)
                if hooks and (c + 1) in hooks:
                    hooks.pop(c + 1)()

            def do_chunk(k_ap, v_ap, bias_ap):
                c = state["c"]
                sts = []
                for qo, qw, tiles in QH:
                    st = stp.tile([128, 1024], F32, tag="st")
                    for o, w in tiles:
                        nc.tensor.matmul(st[:, o - qo:o - qo + w], k_ap,
                                         qbf[slot][:, o:o + w],
                                         start=True, stop=True)
                    sts.append(st)
                ex = ep.tile([128, 2 * 1024], BF16, tag="ex")
                bias = 0.0 if bias_ap is None else bias_ap
                for (qo, qw, tiles), st in zip(QH, sts):
                    nc.scalar.activation(ex[:, qo:qo + qw], st[:, 0:qw],
                                         AF.Exp, bias=bias)
                if state.get("pend") is not None:
                    emit_tail(state["pend"])
                state["pend"] = {"c": c, "ex": ex, "v": v_ap}
                state["c"] += 1

            # dram segment, streamed in SUPER-key groups
            with tc.tile_pool(name=f"ks{slot}", bufs=3) as kp, \
                 tc.tile_pool(name=f"vs{slot}", bufs=3) as vp:
                LD = n_dram * 128
                soff = 0
                while soff < LD:
                    ssz = min(SUPER, LD - soff)
                    ks = kp.tile([128, SUPER], BF16, tag="ks")
                    nc.sync.dma_start(ks[:, 0:ssz], kd.ap()[:, soff:soff + ssz])
                    vs = vp.tile([128, SUPER], BF16, tag="vs")
                    nsub = ssz // 128
                    nc.sync.dma_start(
                        vs[:, 0:ssz].rearrange("p (j d) -> p j d", d=128),
                        vd.ap()[soff:soff + ssz, :].rearrange(
                            "(j p) d -> p j d", p=128))
                    for j in range(nsub):
                        c = state["c"]
                        bcol = bias_cols.get((slot, c))
                        bias_ap = None if bcol is None else biast[:, bcol:bcol + 1]
                        do_chunk(ks[:, j * 128:(j + 1) * 128],
                                 vs[:, j * 128:(j + 1) * 128], bias_ap)
                    soff += ssz
            # sbuf (new keys) segment
            for j in range(n_s):
                c = state["c"]
                bcol = bias_cols.get((slot, c))
                bias_ap = None if bcol is None else biast[:, bcol:bcol + 1]
                do_chunk(kbf[slot][:, j * 128:(j + 1) * 128],
                         v_nat[slot][:, j * 128:(j + 1) * 128], bias_ap)
            emit_tail(state["pend"])
            state["pend"] = None

        def den_reduce(slot, dps_pool):
            for gi in range(0, NQT, 2):
                dps = dps_pool.tile([128, 1024], F32, tag="st")
                for t in range(gi, min(gi + 2, NQT)):
                    qo, qw = QT[t]
                    o = (t - gi) * 512
                    nc.tensor.matmul(dps[0:1, o:o + qw], ones_r[:],
                                     acc_hi[:, qo:qo + qw], start=True,
                                     stop=True)
                    nc.vector.tensor_copy(den_sb[slot][0:1, qo:qo + qw],
                                          dps[0:1, o:o + qw])

        # ---- slot 1 ----
        p1.close()
        attn_sb = ctx.enter_context(tc.tile_pool(name="attn_sb", bufs=1))
        out1_sb = attn_sb.tile([128, T], BF16, tag="out1sb")
        den_sb = [attn_sb.tile([1, T], F32, tag=f"den{s2}", name=f"den{s2}")
                  for s2 in range(2)]
        of0 = attn_sb.tile([128, T], BF16, tag="of0")
        of1 = attn_sb.tile([128, T], BF16, tag="of1")
        with tc.tile_pool(name="ops1", bufs=1, space="PSUM") as ops1:
            out1_ps = [ops1.tile([128, w], F32, tag=f"o1_{t}", name=f"o1_{t}")
                       for t, (o, w) in enumerate(QT)]
            with tc.tile_pool(name="st1", bufs=2, space="PSUM") as stp1, \
                 tc.tile_pool(name="ex1", bufs=5) as ep1:
                attend(1, n_d1, kc1_d, vc1_d, out1_ps, stp1, ep1)
            # evacuate (unnormalized, bf16); den1 reduce deferred into slot 0
            for t, (qo, qw) in enumerate(QT):
                nc.vector.tensor_copy(out1_sb[:, qo:qo + qw],
                                      out1_ps[t][:, 0:qw])

        # ---- slot 0 ----
        with tc.tile_pool(name="finc", bufs=1) as fc_:
            woT_sb = fc_.tile([128, 2 * XD], BF16)
            nc.sync.dma_start(woT_sb[:, 0:XD], woT_d.ap()[0:128, :])
            nc.sync.dma_start(woT_sb[:, XD:2 * XD], woT_d.ap()[128:256, :])

            def emit_of1():
                den1_tot = fc_.tile([1, T], F32, tag="d1t")
                nc.sync.dma_start(den1_tot[:], cbuf["den1"][1][:])
                rc = fc_.tile([1, T], F32, tag="rc1")
                nc.vector.reciprocal_approx_fast(rc[:], den1_tot[:])
                rb = fc_.tile([128, T], F32, tag="rb1")
                nc.gpsimd.partition_broadcast(rb[:], rc[0:1, :])
                for t, (qo, qw) in enumerate(QT):
                    nc.gpsimd.tensor_tensor(
                        out=of1[:, qo:qo + qw], in0=out1_sb[:, qo:qo + qw],
                        in1=rb[:, qo:qo + qw], op=ALU.mult)

            trigger = max(0, nch[0] - 18)

            def emit_den1():
                den_reduce(1, stp0)
                nc.gpsimd.dma_start(cbuf["den1"][0][:], den_sb[1][:])
                nc.gpsimd.collective_compute(
                    "AllReduce", ALU.add, replica_groups=groups_pair,
                    ins=[cbuf["den1"][0].opt()], outs=[cbuf["den1"][1].opt()])

            with tc.tile_pool(name="ops0", bufs=1, space="PSUM") as ops0:
                out0_ps = [ops0.tile([128, w], F32, tag=f"o0_{t}",
                                     name=f"o0_{t}")
                           for t, (o, w) in enumerate(QT)]
                with tc.tile_pool(name="st0", bufs=2, space="PSUM") as stp0, \
                     tc.tile_pool(name="ex0", bufs=5) as ep0:
                    attend(0, n_d0, kc0_d, vc0_d, out0_ps, stp0, ep0,
                           hooks={2: emit_den1, trigger: emit_of1})
                    den_reduce(0, stp0)

                rc0 = fc_.tile([1, T], F32, tag="rc0")
                nc.vector.reciprocal_approx_fast(rc0[:], den_sb[0][:])
                rb0 = fc_.tile([128, T], F32, tag="rb0")
                nc.gpsimd.partition_broadcast(rb0[:], rc0[0:1, :])
                for t, (qo, qw) in enumerate(QT):
                    nc.vector.tensor_tensor(
                        out=of0[:, qo:qo + qw], in0=out0_ps[t][:, 0:qw],
                        in1=rb0[:, qo:qo + qw], op=ALU.mult)

            # ============ P3: output projection ============
            with tc.tile_pool(name="fin", bufs=4) as fp_, \
                 tc.tile_pool(name="yps", bufs=4, space="PSUM") as yps:
                u = 0
                for jc in range(NJ):
                    for hi, (qo, qw, tiles) in enumerate(QH):
                        yp = yps.tile([128, 1024], F32, tag="yp")
                        for o, w in tiles:
                            nc.tensor.matmul(
                                yp[:, o - qo:o - qo + w],
                                woT_sb[:, jc * 128:(jc + 1) * 128],
                                of0[:, o:o + w], start=True, stop=False)
                            nc.tensor.matmul(
                                yp[:, o - qo:o - qo + w],
                                woT_sb[:, XD + jc * 128:XD + (jc + 1) * 128],
                                of1[:, o:o + w], start=False, stop=True)
                        ysb = fp_.tile([128, 1024], BF16, tag="ysb")
                        u += 1
                        nc.vector.tensor_copy(ysb[:, 0:qw], yp[:, 0:qw])
                        nc.sync.dma_start(
                            yT_d.ap()[jc * 128:(jc + 1) * 128, qo:qo + qw],
                            ysb[:, 0:qw])

    nc.compile()
    return nc


# ---------------- host side ----------------

def host_prepare(cfg, x, freqs_cos, freqs_sin, wq, bq, wk, bk, wv, bv,
                 wo, bo, gq, gk, win_old_k, win_old_v, n_cores=8):
    """win_old_k/v: [L0, XD] assembled old window (eviction applied)."""
    import ml_dtypes
    bf16 = ml_dtypes.bfloat16
    T, XD = cfg["T"], cfg["XD"]
    L0 = cfg["L0"]
    NEW = T
    half0 = ceil_div(L0, 2)
    n_d1 = ceil_div(max(half0, L0 - half0), 128)
    L1D = n_d1 * 128
    n_d0 = ceil_div(L0, 128)
    L0D = n_d0 * 128
    n_s = ceil_div(NEW, 128)
    # bias layout must match build_program
    nb = (n_s + 1) + 2

    f32 = np.float32
    xT = np.ascontiguousarray(x.reshape(T, XD).T).astype(bf16)
    cos2 = np.concatenate([freqs_cos.T, freqs_cos.T], 0).astype(f32)
    sin2 = np.concatenate([freqs_sin.T, freqs_sin.T], 0).astype(f32)
    swap_idx = np.r_[64:128, 0:64]

    in_maps = []
    for c in range(n_cores):
        p, role = divmod(c, 2)
        h0 = 3 * p + (0 if role == 0 else 2)
        h1 = 3 * p + 1
        cols = np.r_[h0 * 128:(h0 + 1) * 128, h1 * 128:(h1 + 1) * 128]
        cols0 = np.r_[h0 * 128:(h0 + 1) * 128]
        cols1 = np.r_[h1 * 128:(h1 + 1) * 128]

        m = {"xT": xT}
        m["wq"] = np.ascontiguousarray(wq[cols, :].T).astype(bf16)
        m["wk"] = np.ascontiguousarray(wk[cols, :].T).astype(bf16)
        m["wv"] = np.ascontiguousarray(wv[cols, :].T).astype(bf16)
        m["woT"] = np.ascontiguousarray(wo[:, cols].T).astype(bf16)
        m["bq"] = np.ascontiguousarray(bq[cols][:, None], f32)
        m["bk"] = np.ascontiguousarray(bk[cols][:, None], f32)
        m["bv"] = np.ascontiguousarray(bv[cols][:, None], f32)
        # g folded into cos/sin tables per (nm, cc)
        for nm, g in (("q", gq), ("k", gk)):
            for cc in range(2):
                gc = g[cols[cc * 128:(cc + 1) * 128]]
                m[f"cs_{nm}{cc}c"] = np.ascontiguousarray(cos2 * gc[:, None]).astype(bf16)
                sgn = np.where(np.arange(128) < 64, -1.0, 1.0)[:, None]
                m[f"cs_{nm}{cc}s"] = np.ascontiguousarray(
                    sin2 * gc[swap_idx][:, None] * sgn).astype(bf16)

        sqmask = np.zeros((256, 1), f32)
        sqmask[0:128] = 1.0
        q4 = 32
        if role == 0:
            sqmask[128:128 + q4] = 1.0
            sqmask[128 + 2 * q4:128 + 3 * q4] = 1.0
        else:
            sqmask[128 + q4:128 + 2 * q4] = 1.0
            sqmask[128 + 3 * q4:] = 1.0
        m["sqmask"] = sqmask.astype(bf16)
        m["identb"] = np.eye(128).astype(bf16)
        m["onesc"] = np.ones((128, 1), f32)
        m["onesrow"] = np.ones((1, 128), f32)

        # slot0: full head h0 window
        k0 = np.zeros((128, L0D), bf16)
        k0[:, 0:L0] = win_old_k[:, cols0].T.astype(bf16)
        v0 = np.zeros((L0D, 128), bf16)
        v0[0:L0] = win_old_v[:, cols0].astype(bf16)
        m["kc0T"] = k0
        m["vc0"] = v0

        # slot1: half the cache for h1
        half0_ = half0
        if role == 0:
            lo, hi = 0, half0_
        else:
            lo, hi = half0_, L0
        nvalid1 = hi - lo
        k1 = np.zeros((128, L1D), bf16)
        k1[:, 0:nvalid1] = win_old_k[lo:hi][:, cols1].T.astype(bf16)
        v1 = np.zeros((L1D, 128), bf16)
        v1[0:nvalid1] = win_old_v[lo:hi][:, cols1].astype(bf16)
        m["kc1T"] = k1
        m["vc1"] = v1

        # bias table
        bias = np.zeros((128, nb), f32)
        col = 0
        # slot1 dram last chunk: valid keys in [ (n_d1-1)*128, nvalid1 )
        lv = int(np.clip(nvalid1 - (n_d1 - 1) * 128, 0, 128))
        bias[lv:, col] = NEG_BIAS
        col += 1
        # slot1 sbuf chunks: this core's new-token half
        nlo = 0 if role == 0 else NEW // 2
        nhi = NEW // 2 if role == 0 else NEW
        for j in range(n_s):
            base = j * 128
            valid = np.zeros(128, bool)
            for r in range(128):
                t_ = base + r
                valid[r] = (nlo <= t_ < nhi) and t_ < NEW
            bias[~valid, col] = NEG_BIAS
            col += 1
        # slot0 dram last
        lv = int(np.clip(L0 - (n_d0 - 1) * 128, 0, 128))
        bias[lv:, col] = NEG_BIAS
        col += 1
        # slot0 sbuf last
        lv = int(np.clip(NEW - (n_s - 1) * 128, 0, 128))
        bias[lv:, col] = NEG_BIAS
        col += 1
        m["biast"] = bias
        in_maps.append(m)
    return in_maps


def host_finalize(cfg, results, bo):
    y = None
    for r in results:
        yi = r["yT"].astype(np.float32)
        y = yi if y is None else y + yi
    y = y.T + bo[None, :]
    return np.ascontiguousarray(y)[None].astype(np.float32)  # [1, T, XD]


def numpy_reference(cfg, x, freqs_cos, freqs_sin, wq, bq, wk, bk, wv, bv,
                    wo, bo, gq, gk, win_old_k, win_old_v):
    """Reference for arbitrary cfg: attention over [old window; new]."""
    T, XD, D = cfg["T"], cfg["XD"], cfg["D"]
    H = XD // D
    x2 = x.reshape(T, XD).astype(np.float64)

    def rms(t, g):
        return t / np.sqrt((t ** 2).mean(-1, keepdims=True) + EPS) * g

    q = rms(x2 @ wq.T + bq, gq)
    k = rms(x2 @ wk.T + bk, gk)
    v = x2 @ wv.T + bv

    def rope(t):
        th = t.reshape(T, H, D)
        t1, t2 = th[..., :D // 2], th[..., D // 2:]
        c = freqs_cos[:, None, :]
        s = freqs_sin[:, None, :]
        return np.concatenate([t1 * c - t2 * s, t1 * s + t2 * c],
                              -1).reshape(T, XD)

    rq, rk = rope(q), rope(k)
    kw = np.concatenate([win_old_k, rk], 0).reshape(-1, H, D)
    vw = np.concatenate([win_old_v, v], 0).reshape(-1, H, D)
    qh = rq.reshape(T, H, D)
    scores = np.einsum("thd,shd->hts", qh, kw) / math.sqrt(D)
    e = np.exp(scores - scores.max(-1, keepdims=True))
    probs = e / e.sum(-1, keepdims=True)
    out = np.einsum("hts,shd->thd", probs, vw).reshape(T, XD)
    return (out @ wo.T + bo)[None].astype(np.float32)


# =====================================================================
# kernel() entry point — full inputs in, full output out.
# =====================================================================

import os as _os
import time as _time
from concourse import bass_utils as _bass_utils

_DIM = 1536
_HEADS = 12
_HD = 128
_FRAME = 1560
_LOCAL_ATTN_SIZE = 6
_SINK_SIZE = 1
_CACHE = _LOCAL_ATTN_SIZE * _FRAME
_SINK = _SINK_SIZE * _FRAME
_MAX_ATTN = _LOCAL_ATTN_SIZE * _FRAME
_GLOBAL_END = _CACHE
_LOCAL_END = _CACHE

_prog_cache = {}
last_exec_ns = None
last_wall_ns = None


def _window_index(current_start, T):
    """Mirrors the reference's rolling-cache index math; returns original
    cache row indices of the attention window's old part."""
    cur_end = current_start + T
    if cur_end > _GLOBAL_END and T + _LOCAL_END > _CACHE:
        evict = T + _LOCAL_END - _CACHE
        rolled = _LOCAL_END - evict - _SINK
        le = _LOCAL_END + cur_end - _GLOBAL_END - evict
    else:
        evict, rolled = 0, 0
        le = _LOCAL_END + cur_end - _GLOBAL_END
    ls = le - T
    ws = max(0, le - _MAX_ATTN)
    idx = np.arange(ws, ls)
    if evict:
        shift = (idx >= _SINK) & (idx < _SINK + rolled)
        idx = np.where(shift, idx + evict, idx)
    return idx


def kernel(**inputs):
    global last_exec_ns, last_wall_ns
    x = np.ascontiguousarray(np.asarray(inputs["x"], np.float32))
    B, T, XD = x.shape
    assert B == 1 and XD == _DIM
    cs = int(np.asarray(inputs["current_start"]))
    idx_old = _window_index(cs, T)
    L0 = len(idx_old)
    cfg = dict(T=T, XD=XD, D=_HD, L0=L0, SUPER=512)

    key = tuple(sorted(cfg.items()))
    if key not in _prog_cache:
        _prog_cache[key] = build_program(cfg, n_cores=8)
    nc = _prog_cache[key]

    f32 = np.float32
    ck = np.asarray(inputs["cache_k"], f32)[0]
    cv = np.asarray(inputs["cache_v"], f32)[0]
    win_k = np.ascontiguousarray(ck[idx_old])
    win_v = np.ascontiguousarray(cv[idx_old])
    bo = np.asarray(inputs["bo"], f32)
    args = (x, np.asarray(inputs["freqs_cos"], f32),
            np.asarray(inputs["freqs_sin"], f32),
            np.asarray(inputs["wq"], f32), np.asarray(inputs["bq"], f32),
            np.asarray(inputs["wk"], f32), np.asarray(inputs["bk"], f32),
            np.asarray(inputs["wv"], f32), np.asarray(inputs["bv"], f32),
            np.asarray(inputs["wo"], f32), bo,
            np.asarray(inputs["gq"], f32), np.asarray(inputs["gk"], f32),
            win_k, win_v)
    in_maps = host_prepare(cfg, *args, n_cores=8)

    trace = bool(int(_os.environ.get("WAN_KERNEL_TRACE", "0")))
    t0 = _time.time()
    try:
        res = _bass_utils.run_bass_kernel_spmd(
            nc, in_maps, core_ids=list(range(8)), trace=trace)
    except (ImportError, ModuleNotFoundError):
        res = _bass_utils.run_bass_kernel_spmd(
            nc, in_maps, core_ids=list(range(8)))
    last_wall_ns = int((_time.time() - t0) * 1e9)
    last_exec_ns = res.exec_time_ns
    return host_finalize(cfg, [res.results[c] for c in range(8)], bo)
